# revision 19
# baseline (speedup 1.0000x reference)
"""DGCNN forward kernel for Trainium2 (8 NeuronCores, data-parallel over batch).

Each core processes one point cloud (N=2048 points) end to end:
  4x EdgeConv (KNN k=20 + 1x1 conv + BN + LeakyReLU(0.2) + max over k)
  -> concat -> 1x1 conv to 1024 + BN + LeakyReLU -> global max+mean pool
  -> MLP 2048-512-256-128-2 with LeakyReLU(0.01).

Algebraic rewrite (as baseline): max_k f(W @ [nbr - ctr, ctr]) = f(max_k(U[idx_k]) + V)
with U = Wl @ x, V = (Wr - Wl) @ x.

v2 changes vs baseline (same fp32 trunk numerics, better engine balance):
  - nsq folded into the S matmul as an extra contraction row (lhsT gets a ones
    row, rhs gets the -|x|^2 row) -> halves fp32 S-matmul column streams.
  - xA holds 2*x so the distance matmul needs no separate doubling.
  - index-wrap (selr) matmuls in fp16 (indices < 2048 are exact in fp16).
  - weights pre-transposed/pre-folded on host (wlT/wvT/bn s,t/W4T hi-lo).
  - conv5 in split-bf16 (3 terms) riding under the layer-3 pipeline; pooling
    via monotone max (pre-activation) + Act accumulators for the mean.
  - per-4-tile-group epilogues (fatter matmuls/activations).
"""

import numpy as np
from contextlib import ExitStack

import concourse.bass as bass
import concourse.bacc as bacc
import concourse.tile as tile
from concourse import mybir
from concourse.bass_utils import run_bass_kernel_spmd
from concourse.masks import make_identity

F32 = mybir.dt.float32
F16 = mybir.dt.float16
BF16 = mybir.dt.bfloat16
I16 = mybir.dt.int16
U32 = mybir.dt.uint32
AF = mybir.ActivationFunctionType
ALU = mybir.AluOpType
AX = mybir.AxisListType

B, N, KNN, P = 8, 2048, 20, 128
NT = N // P                      # 16 point tiles
NG = NT // 4                     # 4 groups of 4 tiles
EPS = 1e-5
NEG = -1e30
CONV = [(64, 3), (64, 64), (128, 64), (256, 128)]   # (O, C) of edge convs
LIN = [(512, 2048), (256, 512), (128, 256), (2, 128)]
LRELU_CONV = 0.2
LRELU_HEAD = 0.01


def _bn_fold(nc, sb, g_col, b_col, m_col, v_col, ncols, eps_col):
    """s = g * rsqrt(v + eps); t = b - m * s  (all [128, ncols] column tiles)."""
    s = sb.tile([P, ncols], F32, tag="bn_s")
    t = sb.tile([P, ncols], F32, tag="bn_t")
    tmp = sb.tile([P, ncols], F32, tag="bn_tmp")
    nc.scalar.activation(out=tmp, in_=v_col, func=AF.Sqrt, bias=eps_col, scale=1.0)
    nc.vector.reciprocal(out=s, in_=tmp)
    nc.vector.tensor_mul(s, s, g_col)
    nc.vector.tensor_mul(tmp, m_col, s)
    nc.vector.tensor_sub(t, b_col, tmp)
    return s, t


def _emit(nc, tc, t_in, t_w, t_out, dbg):
    with ExitStack() as ctx:
        const = ctx.enter_context(tc.tile_pool(name="const", bufs=1))
        pers = ctx.enter_context(tc.tile_pool(name="pers", bufs=1))
        ps_s = ctx.enter_context(tc.tile_pool(name="ps_s", bufs=3, space="PSUM"))
        ps_e = ctx.enter_context(tc.tile_pool(name="ps_e", bufs=2, space="PSUM"))
        ps_m = ctx.enter_context(tc.tile_pool(name="ps_m", bufs=2, space="PSUM"))
        mstack = ExitStack()  # closed before the head to free SBUF
        sbs = mstack.enter_context(tc.tile_pool(name="sbs", bufs=2))   # s_sb
        sbw = mstack.enter_context(tc.tile_pool(name="sbw", bufs=2))   # small work tiles
        sbg = mstack.enter_context(tc.tile_pool(name="sbg", bufs=2))   # gather out
        sbx = mstack.enter_context(tc.tile_pool(name="sbx", bufs=1))   # x slots (tagged)

        ident = const.tile([P, P], F32)
        make_identity(nc, ident[:])
        ident16 = const.tile([P, P], F16)
        nc.vector.tensor_copy(out=ident16, in_=ident)
        ones_row = const.tile([1, P], F32)
        nc.vector.memset(ones_row, 1.0)
        ones_col = const.tile([P, 1], F32)
        nc.vector.memset(ones_col, 1.0)
        eps_col = const.tile([P, 1], F32)
        nc.vector.memset(eps_col, EPS)

        # SELR[g][p, p'] = 1 iff p == g*16 + p' % 16  (wrapped-idx builder), fp16
        selr = const.tile([P, 8, P], F16)
        for g in range(8):
            isrc = ident16[:, g * 16:(g + 1) * 16]
            src_b = bass.AP(tensor=isrc.tensor, offset=isrc.offset,
                            ap=[isrc.ap[0], [0, 8], isrc.ap[1]])
            nc.vector.tensor_copy(
                out=selr[:, g, :].rearrange("p (o q) -> p o q", q=16), in_=src_b)

        # persistent f32 layer outputs (conv5 cat operands + next-layer inputs)
        xp = [pers.tile([65, N], F32, name="x0p"),
              pers.tile([65, N], F32, name="x1p"),
              pers.tile([P, N], F32, name="x2p"),
              pers.tile([P, 2 * N], F32, name="x3p")]
        p_cf = pers.tile([P, 16], F32)
        mean_z = pers.tile([P, 8, NG], F32)    # sum of pre-act h per (j, group)
        mean_r = pers.tile([P, 8, NG], F32)    # sum of relu(-h)

        # conv5 weights: W4T chains [crow, 1024] fp32
        chains = [(0, 64, 0), (1, 64, 0), (2, 128, 0), (3, 128, 0), (3, 128, N)]
        # (source xp idx, rows, free offset); W4T row offsets:
        c4off = [0, 64, 128, 256, 384]
        w4c = [pers.tile([crow, 1024], F32, name=f"w4c{ci}")
               for ci, (_, crow, _) in enumerate(chains)]
        for ci, (_, crow, _) in enumerate(chains):
            nc.sync.dma_start(out=w4c[ci], in_=t_w["w4t"][c4off[ci]:c4off[ci] + crow, :])
        s4 = pers.tile([P, 8], F32)
        t4 = pers.tile([P, 8], F32)
        for j in range(8):
            nc.sync.dma_start(out=s4[:, j:j + 1], in_=t_w["bns4"][j * P:(j + 1) * P, :])
            nc.sync.dma_start(out=t4[:, j:j + 1], in_=t_w["bnt4"][j * P:(j + 1) * P, :])

        # ---------------- input transpose: feat [N, 3] -> xB0 [3, N], xA0 = 2x --
        xa0 = sbx.tile([P, N], F32, tag="xA0", name="xA0")
        xb0 = sbx.tile([5, N], F32, tag="xB0", name="xB0")
        nsq0 = xb0[3:4, :]
        for t in range(NT):
            ft = sbw.tile([P, 3], F32, tag="feat")
            nc.sync.dma_start(out=ft, in_=t_in["feat_xyz"][t * P:(t + 1) * P, :])
            pt = ps_m.tile([P, P], F32, tag="m")
            nc.tensor.transpose(out=pt[0:3, 0:P], in_=ft[:, :], identity=ident)
            sl = slice(t * P, (t + 1) * P)
            nc.scalar.activation(out=xb0[0:3, sl], in_=pt[0:3, 0:P], func=AF.Copy)
            nc.scalar.activation(out=xa0[0:3, sl], in_=pt[0:3, 0:P], func=AF.Copy,
                                 scale=2.0)
        nc.sync.dma_start(out=xa0[3:4, :], in_=t_w["onesN"][:, :])

        xa, xb, nsq = xa0, xb0, nsq0
        # =================== edge conv layers ===================
        for li, (O, C) in enumerate(CONV):
            OCH = (O + P - 1) // P
            is3 = (C == P)
            with ExitStack() as lctx:
                sb = lctx.enter_context(tc.tile_pool(name=f"sb_l{li}", bufs=1))
                u_dram = t_w[f"Utab{li}"]

                # --- weights (host-pretransposed)
                wlT = sb.tile([P, O], F32, tag="wlT")
                wvT = sb.tile([P, O], F32, tag="wvT")
                nc.sync.dma_start(out=wlT[0:C, :], in_=t_w[f"wlT{li}"][:, :])
                nc.sync.dma_start(out=wvT[0:C, :], in_=t_w[f"wvT{li}"][:, :])
                bns = sb.tile([P, OCH], F32, tag="bns")
                bnt = sb.tile([P, OCH], F32, tag="bnt")
                for j in range(OCH):
                    ow = min(P, O - j * P)
                    nc.sync.dma_start(out=bns[0:ow, j:j + 1],
                                      in_=t_w[f"bns{li}"][j * P:j * P + ow, :])
                    nc.sync.dma_start(out=bnt[0:ow, j:j + 1],
                                      in_=t_w[f"bnt{li}"][j * P:j * P + ow, :])

                # --- nsq row: -sum_c x^2 (scratch at partition 0, DMA to row C)
                for q in range(4):
                    sl = slice(q * 512, (q + 1) * 512)
                    xxb = sbw.tile([P, 512], F32, tag="xx")
                    nc.scalar.activation(out=xxb[0:C, :], in_=xb[0:C, sl],
                                         func=AF.Square)
                    pq = ps_m.tile([1, 512], F32, tag="m")
                    nc.tensor.matmul(out=pq, lhsT=ones_col[0:C, :], rhs=xxb[0:C, :],
                                     start=True, stop=True)
                    nscr = sbw.tile([1, 512], F32, tag="nsq_scr")
                    nc.scalar.activation(out=nscr, in_=pq, func=AF.Copy, scale=-1.0)
                    nc.sync.dma_start(out=nsq[:, sl], in_=nscr)

                # --- U table -> DRAM
                for t in range(NT):
                    pu = ps_m.tile([P, 512], F32, tag="m")
                    nc.tensor.matmul(out=pu[:, 0:O], lhsT=xb[0:C, t * P:(t + 1) * P],
                                     rhs=wlT[0:C, 0:O], start=True, stop=True)
                    usb = sbw.tile([P, O], F32, tag="u_sb")
                    nc.scalar.activation(out=usb, in_=pu[:, 0:O], func=AF.Copy)
                    nc.sync.dma_start(out=u_dram[t * P:(t + 1) * P, :], in_=usb)

                # next-layer xA slot (xB comes from persistent xp[li])
                if li < 3:
                    nxa = sbx.tile([P, N], F32, tag=f"xA{(li + 1) % 2}",
                                   name=f"xA{li + 1}")
                else:
                    nxa = None
                def epilogue(g, m_grp):
                    """conv epilogue for group g (points g*512:(g+1)*512)."""
                    gsl = slice(g * 512, (g + 1) * 512)
                    for j in range(OCH):
                        ow = min(P, O - j * P)
                        pe = ps_e.tile([P, 512], F32, tag="e_ps")
                        nc.tensor.matmul(out=pe[0:ow, :],
                                         lhsT=wvT[0:C, j * P:j * P + ow],
                                         rhs=xb[0:C, gsl], start=True, stop=False)
                        for tt in range(4):
                            msl = m_grp[:, tt * O + j * P: tt * O + j * P + ow]
                            nc.tensor.matmul(
                                out=pe[0:ow, tt * P:(tt + 1) * P],
                                lhsT=msl, rhs=ident,
                                is_transpose=True, start=False, stop=(tt == 3),
                                skip_group_check=True)
                        # y = lrelu(bn(...)); write to next-layer xB (or scratch for l3)
                        if li < 3:
                            # OCH == 1 always here (O <= 128), so j == 0
                            dst = xp[li][j * P:j * P + ow, gsl]
                        else:
                            dst = xp[3][:, j * N + g * 512:j * N + (g + 1) * 512]
                        nc.scalar.activation(out=dst, in_=pe[0:ow, :],
                                             func=AF.Identity,
                                             scale=bns[0:ow, j:j + 1],
                                             bias=bnt[0:ow, j:j + 1])
                        tmp = sbw.tile([P, 512], F32, tag="lr_tmp")
                        nc.vector.tensor_scalar_mul(tmp[0:ow, :], dst, LRELU_CONV)
                        nc.vector.tensor_tensor(out=dst, in0=dst,
                                                in1=tmp[0:ow, :], op=ALU.max)
                        if li < 3:
                            nc.scalar.activation(out=nxa[j * P:j * P + ow, gsl],
                                                 in_=dst, func=AF.Copy, scale=2.0)

                def conv5(g):
                    """1024-ch conv + pooling for group g (after layer-3 epilogue)."""
                    gsl = slice(g * 512, (g + 1) * 512)
                    for j in range(8):
                        pc = ps_e.tile([P, 512], F32, tag="e_ps")
                        for ci, (lx, crow, fo) in enumerate(chains):
                            fsl = slice(fo + g * 512, fo + (g + 1) * 512)
                            nc.tensor.matmul(
                                out=pc, lhsT=w4c[ci][0:crow, j * P:(j + 1) * P],
                                rhs=xp[lx][0:crow, fsl],
                                start=(ci == 0), stop=(ci == len(chains) - 1))
                        # h pre-act; mean accumulators via two Act passes
                        hs = sbw.tile([P, 512], F32, tag="h_sb")
                        nc.scalar.activation(out=hs, in_=pc, func=AF.Identity,
                                             scale=s4[:, j:j + 1], bias=t4[:, j:j + 1],
                                             accum_out=mean_z[:, j, g:g + 1])
                        hr = sbw.tile([P, 512], F32, tag="lr_tmp")
                        nc.scalar.activation(out=hr, in_=hs, func=AF.Relu, scale=-1.0,
                                             accum_out=mean_r[:, j, g:g + 1])
                        # max-pool on pre-act h (lrelu applied to pooled value later)
                        pm = sbw.tile([P, 1], F32, tag="pmax")
                        nc.vector.tensor_reduce(out=pm, in_=hs, axis=AX.X, op=ALU.max)
                        if g == 0:
                            nc.vector.tensor_copy(out=p_cf[:, j:j + 1], in_=pm)
                        else:
                            nc.vector.tensor_tensor(out=p_cf[:, j:j + 1],
                                                    in0=p_cf[:, j:j + 1], in1=pm,
                                                    op=ALU.max)

                # --- per point-tile: S, top-k, idx wrap, gather, k-reduce
                # S of tile t+1 is emitted before tile t's top-k so the PE
                # computes it during the DVE scans (instead of idling behind
                # the selr matmuls that wait on the top-k).
                def emit_S(t):
                    s_sb = sbs.tile([P, N], F32, tag="s_sb", name="s_sb")
                    for q in range(4):
                        sl = slice(q * 512, (q + 1) * 512)
                        pq = ps_s.tile([P, 512], F32, tag="s_ps")
                        if not is3:
                            nc.tensor.matmul(out=pq,
                                             lhsT=xa[0:C + 1, t * P:(t + 1) * P],
                                             rhs=xb[0:C + 1, sl],
                                             start=True, stop=True)
                        else:
                            nc.tensor.matmul(out=pq,
                                             lhsT=xa[0:C, t * P:(t + 1) * P],
                                             rhs=xb[0:C, sl], start=True, stop=False)
                            nc.tensor.matmul(out=pq, lhsT=ones_row, rhs=nsq[:, sl],
                                             start=False, stop=True)
                        nc.scalar.activation(out=s_sb[:, sl], in_=pq, func=AF.Copy)
                    return s_sb

                pending = []
                epiq = []
                cur_m = [None]
                s_cur = emit_S(0)
                for t in range(NT):
                    if t % 4 == 0:
                        cur_m[0] = sbs.tile([P, 4 * O], F32, tag="m_grp", name="m_grp")
                    s_nxt = emit_S(t + 1) if t + 1 < NT else None
                    s_sb = s_cur
                    v24 = sbw.tile([P, 24], F32, tag="v24")
                    i24 = sbw.tile([P, 24], U32, tag="i24")
                    nc.vector.max(out=v24[:, 0:8], in_=s_sb)
                    nc.vector.max_index(out=i24[:, 0:8], in_max=v24[:, 0:8],
                                        in_values=s_sb)
                    nc.vector.match_replace(out=s_sb, in_to_replace=v24[:, 0:8],
                                            in_values=s_sb, imm_value=NEG)
                    nc.vector.max(out=v24[:, 8:16], in_=s_sb)
                    nc.vector.max_index(out=i24[:, 8:16], in_max=v24[:, 8:16],
                                        in_values=s_sb)
                    nc.vector.match_replace(out=s_sb, in_to_replace=v24[:, 8:16],
                                            in_values=s_sb, imm_value=NEG)
                    nc.vector.max(out=v24[:, 16:24], in_=s_sb)
                    nc.vector.max_index(out=i24[:, 16:24], in_max=v24[:, 16:24],
                                        in_values=s_sb)

                    idxf32 = sbw.tile([P, KNN], F32, tag="idxf32")
                    nc.vector.tensor_copy(out=idxf32, in_=i24[:, 0:KNN])
                    idxf = sbw.tile([P, KNN], F16, tag="idxf")
                    nc.vector.tensor_copy(out=idxf, in_=idxf32)
                    pw = ps_m.tile([P, 8 * KNN], F32, tag="m")
                    for g8 in range(8):
                        nc.tensor.matmul(
                            out=pw[:, :].rearrange("p (k g) -> p k g", g=8)[:, :, g8],
                            lhsT=selr[:, g8, :], rhs=idxf, start=True, stop=True,
                            skip_group_check=True)
                    w16 = sbw.tile([P, 8 * KNN], I16, tag="w16")
                    nc.vector.tensor_copy(out=w16, in_=pw)

                    gt = sbg.tile([P, KNN, O], F32, tag="gather")
                    nc.gpsimd.dma_gather(
                        out_ap=gt[:, :, :], in_ap=u_dram[:, :], idxs_ap=w16[:, :],
                        num_idxs=P * KNN, num_idxs_reg=P * KNN, elem_size=O,
                        single_packet=False)
                    pending.append((t, gt, cur_m[0]))

                    def flush_one():
                        tk, gtk, mgk = pending.pop(0)
                        nc.vector.tensor_reduce(
                            out=mgk[:, (tk % 4) * O:(tk % 4 + 1) * O],
                            in_=gtk[:, :, :].rearrange("p k o -> p o k"),
                            axis=AX.X, op=ALU.max)
                        if tk % 4 == 3:
                            epiq.append((tk // 4, mgk))

                    def drain_epis(now):
                        while epiq and (now or epiq[0][0] * 4 + 6 <= t):
                            g, mgk = epiq.pop(0)
                            epilogue(g, mgk)
                            if li == 3:
                                conv5(g)

                    # lag-1 software pipeline: k-reduce of tile t-1 issues after
                    # tile t's top-k, so the DVE never stalls on the gather.
                    # Epilogues are emitted 2+ tiles later still, so their PE ops
                    # never block upcoming S matmuls on not-yet-passed DVE points.
                    if len(pending) > 1:
                        flush_one()
                    drain_epis(False)
                    if t == NT - 1:
                        while pending:
                            flush_one()
                        drain_epis(True)
                    s_cur = s_nxt
                if li < 3:
                    # ones row of next xA (if next layer has aug row)
                    if CONV[li + 1][1] < P:
                        nc.sync.dma_start(
                            out=nxa[CONV[li + 1][1]:CONV[li + 1][1] + 1, :],
                            in_=t_w["onesN"][:, :])
                if dbg:
                    nc.sync.dma_start(out=t_out[f"dbg_x{li}"][:, :],
                                      in_=xp[li][0:min(O, P), :])
            if li < 3:
                xa = nxa
                xb = xp[li]
                if CONV[li + 1][1] < P:
                    nsq = xp[li][CONV[li + 1][1]:CONV[li + 1][1] + 1, :]
                else:
                    nsq = sbx.tile([1, N], F32, tag="nsq3", name="nsq3")

        # =================== finish pooling ===================
        # mean = (sum_z - 0.8 * sum_relu(-z)) / N ; p_cf[:, 8+j]
        with tc.tile_pool(name="sb_pool", bufs=1) as sbp:
            mz = sbp.tile([P, 8], F32)
            mr = sbp.tile([P, 8], F32)
            nc.vector.tensor_reduce(out=mz, in_=mean_z[:, :, :], axis=AX.X, op=ALU.add)
            nc.vector.tensor_reduce(out=mr, in_=mean_r[:, :, :], axis=AX.X, op=ALU.add)
            # sum lrelu(z) = sum z + (1 - alpha) * sum relu(-z)
            nc.vector.tensor_scalar_mul(mr, mr, 1.0 - LRELU_CONV)
            nc.vector.tensor_add(out=p_cf[:, 8:16], in0=mz, in1=mr)
            nc.vector.tensor_scalar_mul(p_cf[:, 8:16], p_cf[:, 8:16], 1.0 / N)
            # lrelu on max-pooled columns (monotone: lrelu(max) = max(lrelu))
            t8 = sbp.tile([P, 8], F32)
            nc.vector.tensor_scalar_mul(t8, p_cf[:, 0:8], LRELU_CONV)
            nc.vector.tensor_tensor(out=p_cf[:, 0:8], in0=p_cf[:, 0:8], in1=t8,
                                    op=ALU.max)
            if dbg:
                nc.sync.dma_start(out=t_out["dbg_p"][:, :], in_=p_cf[:, :])

        mstack.close()
        # =================== MLP head (broadcast + DVE dot-products) ==========
        with ExitStack() as hctx:
            sb = hctx.enter_context(tc.tile_pool(name="sb_head", bufs=1))
            sbwh = hctx.enter_context(tc.tile_pool(name="sbw_head", bufs=2))

            def lin(name, src_col, incols, w_dram, out_dim, alpha):
                in_dim = P * incols
                och = (out_dim + P - 1) // P
                orows = min(P, out_dim)
                bcast = sb.tile([P, in_dim], F32, tag=f"{name}_bc")
                for j in range(incols):
                    pT = ps_m.tile([1, P], F32, tag="m")
                    nc.tensor.transpose(out=pT, in_=src_col[:, j:j + 1],
                                        identity=ident)
                    rowj = sbwh.tile([1, P], F32, tag="hd_row")
                    nc.scalar.activation(out=rowj, in_=pT, func=AF.Copy)
                    pb = ps_m.tile([P, P], F32, tag="m")
                    nc.tensor.matmul(out=pb, lhsT=ones_row, rhs=rowj,
                                     start=True, stop=True)
                    nc.scalar.activation(out=bcast[:, j * P:(j + 1) * P], in_=pb,
                                         func=AF.Copy)
                dst = sb.tile([P, och], F32, tag=f"{name}_out")
                for ot in range(och):
                    orw = min(P, out_dim - ot * P)
                    wsb = sbwh.tile([P, in_dim], F32, tag=f"{name}_w")
                    nc.sync.dma_start(out=wsb[0:orw, :],
                                      in_=w_dram[ot * P:ot * P + orw, :])
                    prod = sbwh.tile([P, in_dim], F32, tag=f"{name}_prod")
                    nc.vector.tensor_mul(prod[0:orw, :], wsb[0:orw, :], bcast[0:orw, :])
                    nc.vector.tensor_reduce(out=dst[0:orw, ot:ot + 1],
                                            in_=prod[0:orw, :], axis=AX.X, op=ALU.add)
                if alpha is not None:
                    tmp = sbwh.tile([P, och], F32, tag=f"{name}_tmp")
                    nc.vector.tensor_scalar_mul(tmp[0:orows, :], dst[0:orows, :], alpha)
                    nc.vector.tensor_tensor(out=dst[0:orows, :], in0=dst[0:orows, :],
                                            in1=tmp[0:orows, :], op=ALU.max)
                return dst

            y1 = lin("y1", p_cf, 16, t_w["L1"], 512, LRELU_HEAD)
            y2 = lin("y2", y1, 4, t_w["L2"], 256, LRELU_HEAD)
            y3 = lin("y3", y2, 2, t_w["L3"], 128, LRELU_HEAD)
            y4 = lin("y4", y3, 1, t_w["L4"], 2, None)
            osb = sb.tile([2, 1], F32, tag="out_sb")
            nc.vector.tensor_copy(out=osb, in_=y4[0:2, 0:1])
            nc.sync.dma_start(out=t_out["out"][:, :], in_=osb)


_PROG_CACHE = {}


def _build(dbg=False):
    key = ("v2", dbg)
    if key in _PROG_CACHE:
        return _PROG_CACHE[key]
    nc = bacc.Bacc("TRN2", target_bir_lowering=False, debug=False, num_devices=B)
    t_in = {"feat_xyz": nc.declare_dram_parameter("feat_xyz", [N, 3], F32,
                                                  isOutput=False)}
    t_w = {}
    for li, (O, C) in enumerate(CONV):
        t_w[f"wlT{li}"] = nc.declare_dram_parameter(f"wlT{li}", [C, O], F32,
                                                    isOutput=False)
        t_w[f"wvT{li}"] = nc.declare_dram_parameter(f"wvT{li}", [C, O], F32,
                                                    isOutput=False)
        t_w[f"bns{li}"] = nc.declare_dram_parameter(f"bns{li}", [O, 1], F32,
                                                    isOutput=False)
        t_w[f"bnt{li}"] = nc.declare_dram_parameter(f"bnt{li}", [O, 1], F32,
                                                    isOutput=False)
        t_w[f"Utab{li}"] = nc.dram_tensor(f"Utab{li}", [N, O], F32)
    t_w["w4t"] = nc.declare_dram_parameter("w4t", [512, 1024], F32,
                                           isOutput=False)
    t_w["onesN"] = nc.declare_dram_parameter("onesN", [1, N], F32, isOutput=False)
    t_w["bns4"] = nc.declare_dram_parameter("bns4", [1024, 1], F32, isOutput=False)
    t_w["bnt4"] = nc.declare_dram_parameter("bnt4", [1024, 1], F32, isOutput=False)
    for j, (o, c) in enumerate(LIN):
        t_w[f"L{j+1}"] = nc.declare_dram_parameter(f"L{j+1}", [o, c], F32,
                                                   isOutput=False)
    t_out = {"out": nc.declare_dram_parameter("out", [2, 1], F32, isOutput=True)}
    if dbg:
        for li in range(4):
            O = CONV[li][0]
            sh = [P, 2 * N] if O == 256 else [O, N]
            t_out[f"dbg_x{li}"] = nc.declare_dram_parameter(f"dbg_x{li}", sh,
                                                            F32, isOutput=True)
        t_out["dbg_p"] = nc.declare_dram_parameter("dbg_p", [P, 16], F32,
                                                   isOutput=True)

    with tile.TileContext(nc) as tc:
        _emit(nc, tc, t_in, t_w, t_out, dbg)
    nc.compile()
    _PROG_CACHE[key] = nc
    return nc


def _make_in_maps(inputs):
    f32 = lambda a: np.ascontiguousarray(np.asarray(a, np.float32))
    feat = f32(inputs["feat_xyz"])
    common = {}
    for li, (O, C) in enumerate(CONV):
        W = f32(inputs[f"W{li}"])
        wl, wr = W[:, :C], W[:, C:]
        common[f"wlT{li}"] = f32(wl.T)
        common[f"wvT{li}"] = f32((wr - wl).T)
        g, b, m, v = (f32(inputs[f"{n}{li}"]) for n in "gbmv")
        s = g / np.sqrt(v + EPS)
        common[f"bns{li}"] = f32(s.reshape(-1, 1))
        common[f"bnt{li}"] = f32((b - m * s).reshape(-1, 1))
    common["w4t"] = np.ascontiguousarray(f32(inputs["W4"]).T)   # [512, 1024]
    g, b, m, v = (f32(inputs[f"{n}4"]) for n in "gbmv")
    s = g / np.sqrt(v + EPS)
    common["bns4"] = f32(s.reshape(-1, 1))
    common["bnt4"] = f32((b - m * s).reshape(-1, 1))
    common["onesN"] = np.ones((1, N), np.float32)
    for j in range(1, 5):
        common[f"L{j}"] = f32(inputs[f"L{j}"])
    return [dict(common, feat_xyz=np.ascontiguousarray(feat[b])) for b in range(B)]


def run(inputs, dbg=False, trace=False, **kw):
    nc = _build(dbg)
    in_maps = _make_in_maps(inputs)
    return run_bass_kernel_spmd(nc, in_maps, list(range(B)), trace=trace, **kw)


def kernel(**inputs):
    res = run(inputs).results
    out = np.stack([res[b]["out"][:, 0] for b in range(B)], axis=0)
    return out.astype(np.float32)


# revision 20
# speedup vs baseline: 1.0041x; 1.0041x over previous
"""DGCNN forward kernel for Trainium2 (8 NeuronCores, data-parallel over batch).

Each core processes one point cloud (N=2048 points) end to end:
  4x EdgeConv (KNN k=20 + 1x1 conv + BN + LeakyReLU(0.2) + max over k)
  -> concat -> 1x1 conv to 1024 + BN + LeakyReLU -> global max+mean pool
  -> MLP 2048-512-256-128-2 with LeakyReLU(0.01).

Algebraic rewrite (as baseline): max_k f(W @ [nbr - ctr, ctr]) = f(max_k(U[idx_k]) + V)
with U = Wl @ x, V = (Wr - Wl) @ x.

v2 changes vs baseline (same fp32 trunk numerics, better engine balance):
  - nsq folded into the S matmul as an extra contraction row (lhsT gets a ones
    row, rhs gets the -|x|^2 row) -> halves fp32 S-matmul column streams.
  - xA holds 2*x so the distance matmul needs no separate doubling.
  - index-wrap (selr) matmuls in fp16 (indices < 2048 are exact in fp16).
  - weights pre-transposed/pre-folded on host (wlT/wvT/bn s,t/W4T hi-lo).
  - conv5 in split-bf16 (3 terms) riding under the layer-3 pipeline; pooling
    via monotone max (pre-activation) + Act accumulators for the mean.
  - per-4-tile-group epilogues (fatter matmuls/activations).
"""

import numpy as np
from contextlib import ExitStack

import concourse.bass as bass
import concourse.bacc as bacc
import concourse.tile as tile
from concourse import mybir
from concourse.bass_utils import run_bass_kernel_spmd
from concourse.masks import make_identity
from concourse import hw_specs as _hw_specs

# The stock cost model assumes 0.34ns/descriptor for SWDGE (software DGE)
# descriptor generation; measured hardware cost for dma_gather is ~8ns/desc
# (20.4us per 2560-index gather). With the stock value the tile scheduler
# believes gathers are ~2us and schedules their consumers (k-reduce,
# epilogue matmuls) immediately behind them, stalling every engine queue for
# ~20us per tile. Correcting the constant lets the scheduler overlap the
# gathers with independent work.
_hw_specs.TRN2Spec.SWDGE_NS_PER_DESCRIPTOR = 8.0

F32 = mybir.dt.float32
F16 = mybir.dt.float16
BF16 = mybir.dt.bfloat16
I16 = mybir.dt.int16
U32 = mybir.dt.uint32
AF = mybir.ActivationFunctionType
ALU = mybir.AluOpType
AX = mybir.AxisListType

B, N, KNN, P = 8, 2048, 20, 128
NT = N // P                      # 16 point tiles
NG = NT // 4                     # 4 groups of 4 tiles
EPS = 1e-5
NEG = -1e30
CONV = [(64, 3), (64, 64), (128, 64), (256, 128)]   # (O, C) of edge convs
LIN = [(512, 2048), (256, 512), (128, 256), (2, 128)]
LRELU_CONV = 0.2
LRELU_HEAD = 0.01


def _bn_fold(nc, sb, g_col, b_col, m_col, v_col, ncols, eps_col):
    """s = g * rsqrt(v + eps); t = b - m * s  (all [128, ncols] column tiles)."""
    s = sb.tile([P, ncols], F32, tag="bn_s")
    t = sb.tile([P, ncols], F32, tag="bn_t")
    tmp = sb.tile([P, ncols], F32, tag="bn_tmp")
    nc.scalar.activation(out=tmp, in_=v_col, func=AF.Sqrt, bias=eps_col, scale=1.0)
    nc.vector.reciprocal(out=s, in_=tmp)
    nc.vector.tensor_mul(s, s, g_col)
    nc.vector.tensor_mul(tmp, m_col, s)
    nc.vector.tensor_sub(t, b_col, tmp)
    return s, t


def _emit(nc, tc, t_in, t_w, t_out, dbg):
    with ExitStack() as ctx:
        const = ctx.enter_context(tc.tile_pool(name="const", bufs=1))
        pers = ctx.enter_context(tc.tile_pool(name="pers", bufs=1))
        ps_s = ctx.enter_context(tc.tile_pool(name="ps_s", bufs=3, space="PSUM"))
        ps_e = ctx.enter_context(tc.tile_pool(name="ps_e", bufs=2, space="PSUM"))
        ps_m = ctx.enter_context(tc.tile_pool(name="ps_m", bufs=2, space="PSUM"))
        mstack = ExitStack()  # closed before the head to free SBUF
        sbs = mstack.enter_context(tc.tile_pool(name="sbs", bufs=2))   # s_sb
        sbw = mstack.enter_context(tc.tile_pool(name="sbw", bufs=2))   # small work tiles
        sbg = mstack.enter_context(tc.tile_pool(name="sbg", bufs=2))   # gather out
        sbx = mstack.enter_context(tc.tile_pool(name="sbx", bufs=1))   # x slots (tagged)

        ident = const.tile([P, P], F32)
        make_identity(nc, ident[:])
        ident16 = const.tile([P, P], F16)
        nc.vector.tensor_copy(out=ident16, in_=ident)
        ones_row = const.tile([1, P], F32)
        nc.vector.memset(ones_row, 1.0)
        ones_col = const.tile([P, 1], F32)
        nc.vector.memset(ones_col, 1.0)
        eps_col = const.tile([P, 1], F32)
        nc.vector.memset(eps_col, EPS)

        # SELR[g][p, p'] = 1 iff p == g*16 + p' % 16  (wrapped-idx builder), fp16
        selr = const.tile([P, 8, P], F16)
        for g in range(8):
            isrc = ident16[:, g * 16:(g + 1) * 16]
            src_b = bass.AP(tensor=isrc.tensor, offset=isrc.offset,
                            ap=[isrc.ap[0], [0, 8], isrc.ap[1]])
            nc.vector.tensor_copy(
                out=selr[:, g, :].rearrange("p (o q) -> p o q", q=16), in_=src_b)

        # persistent f32 layer outputs (conv5 cat operands + next-layer inputs)
        xp = [pers.tile([65, N], F32, name="x0p"),
              pers.tile([65, N], F32, name="x1p"),
              pers.tile([P, N], F32, name="x2p"),
              pers.tile([P, 2 * N], F32, name="x3p")]
        p_cf = pers.tile([P, 16], F32)
        mean_z = pers.tile([P, 8, NG], F32)    # sum of pre-act h per (j, group)
        mean_r = pers.tile([P, 8, NG], F32)    # sum of relu(-h)

        # conv5 weights: W4T chains [crow, 1024] fp32
        chains = [(0, 64, 0), (1, 64, 0), (2, 128, 0), (3, 128, 0), (3, 128, N)]
        # (source xp idx, rows, free offset); W4T row offsets:
        c4off = [0, 64, 128, 256, 384]
        w4c = [pers.tile([crow, 1024], F32, name=f"w4c{ci}")
               for ci, (_, crow, _) in enumerate(chains)]
        for ci, (_, crow, _) in enumerate(chains):
            nc.sync.dma_start(out=w4c[ci], in_=t_w["w4t"][c4off[ci]:c4off[ci] + crow, :])
        s4 = pers.tile([P, 8], F32)
        t4 = pers.tile([P, 8], F32)
        for j in range(8):
            nc.sync.dma_start(out=s4[:, j:j + 1], in_=t_w["bns4"][j * P:(j + 1) * P, :])
            nc.sync.dma_start(out=t4[:, j:j + 1], in_=t_w["bnt4"][j * P:(j + 1) * P, :])

        # ---------------- input transpose: feat [N, 3] -> xB0 [3, N], xA0 = 2x --
        xa0 = sbx.tile([P, N], F32, tag="xA0", name="xA0")
        xb0 = sbx.tile([5, N], F32, tag="xB0", name="xB0")
        nsq0 = xb0[3:4, :]
        for t in range(NT):
            ft = sbw.tile([P, 3], F32, tag="feat")
            nc.sync.dma_start(out=ft, in_=t_in["feat_xyz"][t * P:(t + 1) * P, :])
            pt = ps_m.tile([P, P], F32, tag="m")
            nc.tensor.transpose(out=pt[0:3, 0:P], in_=ft[:, :], identity=ident)
            sl = slice(t * P, (t + 1) * P)
            nc.scalar.activation(out=xb0[0:3, sl], in_=pt[0:3, 0:P], func=AF.Copy)
            nc.scalar.activation(out=xa0[0:3, sl], in_=pt[0:3, 0:P], func=AF.Copy,
                                 scale=2.0)
        nc.sync.dma_start(out=xa0[3:4, :], in_=t_w["onesN"][:, :])

        xa, xb, nsq = xa0, xb0, nsq0
        # =================== edge conv layers ===================
        for li, (O, C) in enumerate(CONV):
            OCH = (O + P - 1) // P
            is3 = (C == P)
            with ExitStack() as lctx:
                sb = lctx.enter_context(tc.tile_pool(name=f"sb_l{li}", bufs=1))
                u_dram = t_w[f"Utab{li}"]

                # --- weights (host-pretransposed)
                wlT = sb.tile([P, O], F32, tag="wlT")
                wvT = sb.tile([P, O], F32, tag="wvT")
                nc.sync.dma_start(out=wlT[0:C, :], in_=t_w[f"wlT{li}"][:, :])
                nc.sync.dma_start(out=wvT[0:C, :], in_=t_w[f"wvT{li}"][:, :])
                bns = sb.tile([P, OCH], F32, tag="bns")
                bnt = sb.tile([P, OCH], F32, tag="bnt")
                for j in range(OCH):
                    ow = min(P, O - j * P)
                    nc.sync.dma_start(out=bns[0:ow, j:j + 1],
                                      in_=t_w[f"bns{li}"][j * P:j * P + ow, :])
                    nc.sync.dma_start(out=bnt[0:ow, j:j + 1],
                                      in_=t_w[f"bnt{li}"][j * P:j * P + ow, :])

                # --- nsq row: -sum_c x^2 (scratch at partition 0, DMA to row C)
                for q in range(4):
                    sl = slice(q * 512, (q + 1) * 512)
                    xxb = sbw.tile([P, 512], F32, tag="xx")
                    nc.scalar.activation(out=xxb[0:C, :], in_=xb[0:C, sl],
                                         func=AF.Square)
                    pq = ps_m.tile([1, 512], F32, tag="m")
                    nc.tensor.matmul(out=pq, lhsT=ones_col[0:C, :], rhs=xxb[0:C, :],
                                     start=True, stop=True)
                    nscr = sbw.tile([1, 512], F32, tag="nsq_scr")
                    nc.scalar.activation(out=nscr, in_=pq, func=AF.Copy, scale=-1.0)
                    nc.sync.dma_start(out=nsq[:, sl], in_=nscr)

                # --- U table -> DRAM
                for t in range(NT):
                    pu = ps_m.tile([P, 512], F32, tag="m")
                    nc.tensor.matmul(out=pu[:, 0:O], lhsT=xb[0:C, t * P:(t + 1) * P],
                                     rhs=wlT[0:C, 0:O], start=True, stop=True)
                    usb = sbw.tile([P, O], F32, tag="u_sb")
                    nc.scalar.activation(out=usb, in_=pu[:, 0:O], func=AF.Copy)
                    nc.sync.dma_start(out=u_dram[t * P:(t + 1) * P, :], in_=usb)

                # next-layer xA slot (xB comes from persistent xp[li])
                if li < 3:
                    nxa = sbx.tile([P, N], F32, tag=f"xA{(li + 1) % 2}",
                                   name=f"xA{li + 1}")
                else:
                    nxa = None
                def epilogue(g, m_grp):
                    """conv epilogue for group g (points g*512:(g+1)*512)."""
                    gsl = slice(g * 512, (g + 1) * 512)
                    for j in range(OCH):
                        ow = min(P, O - j * P)
                        pe = ps_e.tile([P, 512], F32, tag="e_ps")
                        nc.tensor.matmul(out=pe[0:ow, :],
                                         lhsT=wvT[0:C, j * P:j * P + ow],
                                         rhs=xb[0:C, gsl], start=True, stop=False)
                        for tt in range(4):
                            msl = m_grp[:, tt * O + j * P: tt * O + j * P + ow]
                            nc.tensor.matmul(
                                out=pe[0:ow, tt * P:(tt + 1) * P],
                                lhsT=msl, rhs=ident,
                                is_transpose=True, start=False, stop=(tt == 3),
                                skip_group_check=True)
                        # y = lrelu(bn(...)); write to next-layer xB (or scratch for l3)
                        if li < 3:
                            # OCH == 1 always here (O <= 128), so j == 0
                            dst = xp[li][j * P:j * P + ow, gsl]
                        else:
                            dst = xp[3][:, j * N + g * 512:j * N + (g + 1) * 512]
                        nc.scalar.activation(out=dst, in_=pe[0:ow, :],
                                             func=AF.Identity,
                                             scale=bns[0:ow, j:j + 1],
                                             bias=bnt[0:ow, j:j + 1])
                        tmp = sbw.tile([P, 512], F32, tag="lr_tmp")
                        nc.vector.tensor_scalar_mul(tmp[0:ow, :], dst, LRELU_CONV)
                        nc.vector.tensor_tensor(out=dst, in0=dst,
                                                in1=tmp[0:ow, :], op=ALU.max)
                        if li < 3:
                            nc.scalar.activation(out=nxa[j * P:j * P + ow, gsl],
                                                 in_=dst, func=AF.Copy, scale=2.0)

                def conv5(g):
                    """1024-ch conv + pooling for group g (after layer-3 epilogue)."""
                    gsl = slice(g * 512, (g + 1) * 512)
                    for j in range(8):
                        pc = ps_e.tile([P, 512], F32, tag="e_ps")
                        for ci, (lx, crow, fo) in enumerate(chains):
                            fsl = slice(fo + g * 512, fo + (g + 1) * 512)
                            nc.tensor.matmul(
                                out=pc, lhsT=w4c[ci][0:crow, j * P:(j + 1) * P],
                                rhs=xp[lx][0:crow, fsl],
                                start=(ci == 0), stop=(ci == len(chains) - 1))
                        # h pre-act; mean accumulators via two Act passes
                        hs = sbw.tile([P, 512], F32, tag="h_sb")
                        nc.scalar.activation(out=hs, in_=pc, func=AF.Identity,
                                             scale=s4[:, j:j + 1], bias=t4[:, j:j + 1],
                                             accum_out=mean_z[:, j, g:g + 1])
                        hr = sbw.tile([P, 512], F32, tag="lr_tmp")
                        nc.scalar.activation(out=hr, in_=hs, func=AF.Relu, scale=-1.0,
                                             accum_out=mean_r[:, j, g:g + 1])
                        # max-pool on pre-act h (lrelu applied to pooled value later)
                        pm = sbw.tile([P, 1], F32, tag="pmax")
                        nc.vector.tensor_reduce(out=pm, in_=hs, axis=AX.X, op=ALU.max)
                        if g == 0:
                            nc.vector.tensor_copy(out=p_cf[:, j:j + 1], in_=pm)
                        else:
                            nc.vector.tensor_tensor(out=p_cf[:, j:j + 1],
                                                    in0=p_cf[:, j:j + 1], in1=pm,
                                                    op=ALU.max)

                # --- per point-tile: S, top-k, idx wrap, gather, k-reduce
                # S of tile t+1 is emitted before tile t's top-k so the PE
                # computes it during the DVE scans (instead of idling behind
                # the selr matmuls that wait on the top-k).
                def emit_S(t):
                    s_sb = sbs.tile([P, N], F32, tag="s_sb", name="s_sb")
                    for q in range(4):
                        sl = slice(q * 512, (q + 1) * 512)
                        pq = ps_s.tile([P, 512], F32, tag="s_ps")
                        if not is3:
                            nc.tensor.matmul(out=pq,
                                             lhsT=xa[0:C + 1, t * P:(t + 1) * P],
                                             rhs=xb[0:C + 1, sl],
                                             start=True, stop=True)
                        else:
                            nc.tensor.matmul(out=pq,
                                             lhsT=xa[0:C, t * P:(t + 1) * P],
                                             rhs=xb[0:C, sl], start=True, stop=False)
                            nc.tensor.matmul(out=pq, lhsT=ones_row, rhs=nsq[:, sl],
                                             start=False, stop=True)
                        nc.scalar.activation(out=s_sb[:, sl], in_=pq, func=AF.Copy)
                    return s_sb

                pending = []
                epiq = []
                cur_m = [None]
                s_cur = emit_S(0)
                for t in range(NT):
                    if t % 4 == 0:
                        cur_m[0] = sbs.tile([P, 4 * O], F32, tag="m_grp", name="m_grp")
                    s_nxt = emit_S(t + 1) if t + 1 < NT else None
                    s_sb = s_cur
                    v24 = sbw.tile([P, 24], F32, tag="v24")
                    i24 = sbw.tile([P, 24], U32, tag="i24")
                    nc.vector.max(out=v24[:, 0:8], in_=s_sb)
                    nc.vector.max_index(out=i24[:, 0:8], in_max=v24[:, 0:8],
                                        in_values=s_sb)
                    nc.vector.match_replace(out=s_sb, in_to_replace=v24[:, 0:8],
                                            in_values=s_sb, imm_value=NEG)
                    nc.vector.max(out=v24[:, 8:16], in_=s_sb)
                    nc.vector.max_index(out=i24[:, 8:16], in_max=v24[:, 8:16],
                                        in_values=s_sb)
                    nc.vector.match_replace(out=s_sb, in_to_replace=v24[:, 8:16],
                                            in_values=s_sb, imm_value=NEG)
                    nc.vector.max(out=v24[:, 16:24], in_=s_sb)
                    nc.vector.max_index(out=i24[:, 16:24], in_max=v24[:, 16:24],
                                        in_values=s_sb)

                    idxf32 = sbw.tile([P, KNN], F32, tag="idxf32")
                    nc.vector.tensor_copy(out=idxf32, in_=i24[:, 0:KNN])
                    idxf = sbw.tile([P, KNN], F16, tag="idxf")
                    nc.vector.tensor_copy(out=idxf, in_=idxf32)
                    pw = ps_m.tile([P, 8 * KNN], F32, tag="m")
                    for g8 in range(8):
                        nc.tensor.matmul(
                            out=pw[:, :].rearrange("p (k g) -> p k g", g=8)[:, :, g8],
                            lhsT=selr[:, g8, :], rhs=idxf, start=True, stop=True,
                            skip_group_check=True)
                    w16 = sbw.tile([P, 8 * KNN], I16, tag="w16")
                    nc.vector.tensor_copy(out=w16, in_=pw)

                    gt = sbg.tile([P, KNN, O], F32, tag="gather")
                    nc.gpsimd.dma_gather(
                        out_ap=gt[:, :, :], in_ap=u_dram[:, :], idxs_ap=w16[:, :],
                        num_idxs=P * KNN, num_idxs_reg=P * KNN, elem_size=O,
                        single_packet=False)
                    pending.append((t, gt, cur_m[0]))

                    def flush_one():
                        tk, gtk, mgk = pending.pop(0)
                        nc.vector.tensor_reduce(
                            out=mgk[:, (tk % 4) * O:(tk % 4 + 1) * O],
                            in_=gtk[:, :, :].rearrange("p k o -> p o k"),
                            axis=AX.X, op=ALU.max)
                        if tk % 4 == 3:
                            epiq.append((tk // 4, mgk))

                    def drain_epis(now):
                        while epiq and (now or epiq[0][0] * 4 + 6 <= t):
                            g, mgk = epiq.pop(0)
                            epilogue(g, mgk)
                            if li == 3:
                                conv5(g)

                    # lag-1 software pipeline: k-reduce of tile t-1 issues after
                    # tile t's top-k, so the DVE never stalls on the gather.
                    # Epilogues are emitted 2+ tiles later still, so their PE ops
                    # never block upcoming S matmuls on not-yet-passed DVE points.
                    if len(pending) > 1:
                        flush_one()
                    drain_epis(False)
                    if t == NT - 1:
                        while pending:
                            flush_one()
                        drain_epis(True)
                    s_cur = s_nxt
                if li < 3:
                    # ones row of next xA (if next layer has aug row)
                    if CONV[li + 1][1] < P:
                        nc.sync.dma_start(
                            out=nxa[CONV[li + 1][1]:CONV[li + 1][1] + 1, :],
                            in_=t_w["onesN"][:, :])
                if dbg:
                    nc.sync.dma_start(out=t_out[f"dbg_x{li}"][:, :],
                                      in_=xp[li][0:min(O, P), :])
            if li < 3:
                xa = nxa
                xb = xp[li]
                if CONV[li + 1][1] < P:
                    nsq = xp[li][CONV[li + 1][1]:CONV[li + 1][1] + 1, :]
                else:
                    nsq = sbx.tile([1, N], F32, tag="nsq3", name="nsq3")

        # =================== finish pooling ===================
        # mean = (sum_z - 0.8 * sum_relu(-z)) / N ; p_cf[:, 8+j]
        with tc.tile_pool(name="sb_pool", bufs=1) as sbp:
            mz = sbp.tile([P, 8], F32)
            mr = sbp.tile([P, 8], F32)
            nc.vector.tensor_reduce(out=mz, in_=mean_z[:, :, :], axis=AX.X, op=ALU.add)
            nc.vector.tensor_reduce(out=mr, in_=mean_r[:, :, :], axis=AX.X, op=ALU.add)
            # sum lrelu(z) = sum z + (1 - alpha) * sum relu(-z)
            nc.vector.tensor_scalar_mul(mr, mr, 1.0 - LRELU_CONV)
            nc.vector.tensor_add(out=p_cf[:, 8:16], in0=mz, in1=mr)
            nc.vector.tensor_scalar_mul(p_cf[:, 8:16], p_cf[:, 8:16], 1.0 / N)
            # lrelu on max-pooled columns (monotone: lrelu(max) = max(lrelu))
            t8 = sbp.tile([P, 8], F32)
            nc.vector.tensor_scalar_mul(t8, p_cf[:, 0:8], LRELU_CONV)
            nc.vector.tensor_tensor(out=p_cf[:, 0:8], in0=p_cf[:, 0:8], in1=t8,
                                    op=ALU.max)
            if dbg:
                nc.sync.dma_start(out=t_out["dbg_p"][:, :], in_=p_cf[:, :])

        mstack.close()
        # =================== MLP head (broadcast + DVE dot-products) ==========
        with ExitStack() as hctx:
            sb = hctx.enter_context(tc.tile_pool(name="sb_head", bufs=1))
            sbwh = hctx.enter_context(tc.tile_pool(name="sbw_head", bufs=2))

            def lin(name, src_col, incols, w_dram, out_dim, alpha):
                in_dim = P * incols
                och = (out_dim + P - 1) // P
                orows = min(P, out_dim)
                bcast = sb.tile([P, in_dim], F32, tag=f"{name}_bc")
                for j in range(incols):
                    pT = ps_m.tile([1, P], F32, tag="m")
                    nc.tensor.transpose(out=pT, in_=src_col[:, j:j + 1],
                                        identity=ident)
                    rowj = sbwh.tile([1, P], F32, tag="hd_row")
                    nc.scalar.activation(out=rowj, in_=pT, func=AF.Copy)
                    pb = ps_m.tile([P, P], F32, tag="m")
                    nc.tensor.matmul(out=pb, lhsT=ones_row, rhs=rowj,
                                     start=True, stop=True)
                    nc.scalar.activation(out=bcast[:, j * P:(j + 1) * P], in_=pb,
                                         func=AF.Copy)
                dst = sb.tile([P, och], F32, tag=f"{name}_out")
                for ot in range(och):
                    orw = min(P, out_dim - ot * P)
                    wsb = sbwh.tile([P, in_dim], F32, tag=f"{name}_w")
                    nc.sync.dma_start(out=wsb[0:orw, :],
                                      in_=w_dram[ot * P:ot * P + orw, :])
                    prod = sbwh.tile([P, in_dim], F32, tag=f"{name}_prod")
                    nc.vector.tensor_mul(prod[0:orw, :], wsb[0:orw, :], bcast[0:orw, :])
                    nc.vector.tensor_reduce(out=dst[0:orw, ot:ot + 1],
                                            in_=prod[0:orw, :], axis=AX.X, op=ALU.add)
                if alpha is not None:
                    tmp = sbwh.tile([P, och], F32, tag=f"{name}_tmp")
                    nc.vector.tensor_scalar_mul(tmp[0:orows, :], dst[0:orows, :], alpha)
                    nc.vector.tensor_tensor(out=dst[0:orows, :], in0=dst[0:orows, :],
                                            in1=tmp[0:orows, :], op=ALU.max)
                return dst

            y1 = lin("y1", p_cf, 16, t_w["L1"], 512, LRELU_HEAD)
            y2 = lin("y2", y1, 4, t_w["L2"], 256, LRELU_HEAD)
            y3 = lin("y3", y2, 2, t_w["L3"], 128, LRELU_HEAD)
            y4 = lin("y4", y3, 1, t_w["L4"], 2, None)
            osb = sb.tile([2, 1], F32, tag="out_sb")
            nc.vector.tensor_copy(out=osb, in_=y4[0:2, 0:1])
            nc.sync.dma_start(out=t_out["out"][:, :], in_=osb)


_PROG_CACHE = {}


def _build(dbg=False):
    key = ("v2", dbg)
    if key in _PROG_CACHE:
        return _PROG_CACHE[key]
    nc = bacc.Bacc("TRN2", target_bir_lowering=False, debug=False, num_devices=B)
    t_in = {"feat_xyz": nc.declare_dram_parameter("feat_xyz", [N, 3], F32,
                                                  isOutput=False)}
    t_w = {}
    for li, (O, C) in enumerate(CONV):
        t_w[f"wlT{li}"] = nc.declare_dram_parameter(f"wlT{li}", [C, O], F32,
                                                    isOutput=False)
        t_w[f"wvT{li}"] = nc.declare_dram_parameter(f"wvT{li}", [C, O], F32,
                                                    isOutput=False)
        t_w[f"bns{li}"] = nc.declare_dram_parameter(f"bns{li}", [O, 1], F32,
                                                    isOutput=False)
        t_w[f"bnt{li}"] = nc.declare_dram_parameter(f"bnt{li}", [O, 1], F32,
                                                    isOutput=False)
        t_w[f"Utab{li}"] = nc.dram_tensor(f"Utab{li}", [N, O], F32)
    t_w["w4t"] = nc.declare_dram_parameter("w4t", [512, 1024], F32,
                                           isOutput=False)
    t_w["onesN"] = nc.declare_dram_parameter("onesN", [1, N], F32, isOutput=False)
    t_w["bns4"] = nc.declare_dram_parameter("bns4", [1024, 1], F32, isOutput=False)
    t_w["bnt4"] = nc.declare_dram_parameter("bnt4", [1024, 1], F32, isOutput=False)
    for j, (o, c) in enumerate(LIN):
        t_w[f"L{j+1}"] = nc.declare_dram_parameter(f"L{j+1}", [o, c], F32,
                                                   isOutput=False)
    t_out = {"out": nc.declare_dram_parameter("out", [2, 1], F32, isOutput=True)}
    if dbg:
        for li in range(4):
            O = CONV[li][0]
            sh = [P, 2 * N] if O == 256 else [O, N]
            t_out[f"dbg_x{li}"] = nc.declare_dram_parameter(f"dbg_x{li}", sh,
                                                            F32, isOutput=True)
        t_out["dbg_p"] = nc.declare_dram_parameter("dbg_p", [P, 16], F32,
                                                   isOutput=True)

    with tile.TileContext(nc) as tc:
        _emit(nc, tc, t_in, t_w, t_out, dbg)
    nc.compile()
    _PROG_CACHE[key] = nc
    return nc


def _make_in_maps(inputs):
    f32 = lambda a: np.ascontiguousarray(np.asarray(a, np.float32))
    feat = f32(inputs["feat_xyz"])
    common = {}
    for li, (O, C) in enumerate(CONV):
        W = f32(inputs[f"W{li}"])
        wl, wr = W[:, :C], W[:, C:]
        common[f"wlT{li}"] = f32(wl.T)
        common[f"wvT{li}"] = f32((wr - wl).T)
        g, b, m, v = (f32(inputs[f"{n}{li}"]) for n in "gbmv")
        s = g / np.sqrt(v + EPS)
        common[f"bns{li}"] = f32(s.reshape(-1, 1))
        common[f"bnt{li}"] = f32((b - m * s).reshape(-1, 1))
    common["w4t"] = np.ascontiguousarray(f32(inputs["W4"]).T)   # [512, 1024]
    g, b, m, v = (f32(inputs[f"{n}4"]) for n in "gbmv")
    s = g / np.sqrt(v + EPS)
    common["bns4"] = f32(s.reshape(-1, 1))
    common["bnt4"] = f32((b - m * s).reshape(-1, 1))
    common["onesN"] = np.ones((1, N), np.float32)
    for j in range(1, 5):
        common[f"L{j}"] = f32(inputs[f"L{j}"])
    return [dict(common, feat_xyz=np.ascontiguousarray(feat[b])) for b in range(B)]


def run(inputs, dbg=False, trace=False, **kw):
    nc = _build(dbg)
    in_maps = _make_in_maps(inputs)
    return run_bass_kernel_spmd(nc, in_maps, list(range(B)), trace=trace, **kw)


def kernel(**inputs):
    res = run(inputs).results
    out = np.stack([res[b]["out"][:, 0] for b in range(B)], axis=0)
    return out.astype(np.float32)


# revision 21
# speedup vs baseline: 1.1068x; 1.1023x over previous
"""DGCNN forward kernel for Trainium2 (8 NeuronCores, data-parallel over batch).

Each core processes one point cloud (N=2048 points) end to end:
  4x EdgeConv (KNN k=20 + 1x1 conv + BN + LeakyReLU(0.2) + max over k)
  -> concat -> 1x1 conv to 1024 + BN + LeakyReLU -> global max+mean pool
  -> MLP 2048-512-256-128-2 with LeakyReLU(0.01).

Algebraic rewrite (as baseline): max_k f(W @ [nbr - ctr, ctr]) = f(max_k(U[idx_k]) + V)
with U = Wl @ x, V = (Wr - Wl) @ x.

v2 changes vs baseline (same fp32 trunk numerics, better engine balance):
  - nsq folded into the S matmul as an extra contraction row (lhsT gets a ones
    row, rhs gets the -|x|^2 row) -> halves fp32 S-matmul column streams.
  - xA holds 2*x so the distance matmul needs no separate doubling.
  - index-wrap (selr) matmuls in fp16 (indices < 2048 are exact in fp16).
  - weights pre-transposed/pre-folded on host (wlT/wvT/bn s,t/W4T hi-lo).
  - conv5 in split-bf16 (3 terms) riding under the layer-3 pipeline; pooling
    via monotone max (pre-activation) + Act accumulators for the mean.
  - per-4-tile-group epilogues (fatter matmuls/activations).
"""

import numpy as np
from contextlib import ExitStack

import concourse.bass as bass
import concourse.bacc as bacc
import concourse.tile as tile
from concourse import mybir
from concourse.bass_utils import run_bass_kernel_spmd
from concourse.masks import make_identity
from concourse import hw_specs as _hw_specs

# The stock cost model assumes 0.34ns/descriptor for SWDGE (software DGE)
# descriptor generation; measured hardware cost for dma_gather is ~8ns/desc
# (20.4us per 2560-index gather). With the stock value the tile scheduler
# believes gathers are ~2us and schedules their consumers (k-reduce,
# epilogue matmuls) immediately behind them, stalling every engine queue for
# ~20us per tile. Correcting the constant lets the scheduler overlap the
# gathers with independent work.
_hw_specs.TRN2Spec.SWDGE_NS_PER_DESCRIPTOR = 8.0

F32 = mybir.dt.float32
F16 = mybir.dt.float16
BF16 = mybir.dt.bfloat16
I16 = mybir.dt.int16
U32 = mybir.dt.uint32
AF = mybir.ActivationFunctionType
ALU = mybir.AluOpType
AX = mybir.AxisListType

B, N, KNN, P = 8, 2048, 20, 128
NT = N // P                      # 16 point tiles
NG = NT // 4                     # 4 groups of 4 tiles
EPS = 1e-5
NEG = -1e30
CONV = [(64, 3), (64, 64), (128, 64), (256, 128)]   # (O, C) of edge convs
LIN = [(512, 2048), (256, 512), (128, 256), (2, 128)]
LRELU_CONV = 0.2
LRELU_HEAD = 0.01


def _bn_fold(nc, sb, g_col, b_col, m_col, v_col, ncols, eps_col):
    """s = g * rsqrt(v + eps); t = b - m * s  (all [128, ncols] column tiles)."""
    s = sb.tile([P, ncols], F32, tag="bn_s")
    t = sb.tile([P, ncols], F32, tag="bn_t")
    tmp = sb.tile([P, ncols], F32, tag="bn_tmp")
    nc.scalar.activation(out=tmp, in_=v_col, func=AF.Sqrt, bias=eps_col, scale=1.0)
    nc.vector.reciprocal(out=s, in_=tmp)
    nc.vector.tensor_mul(s, s, g_col)
    nc.vector.tensor_mul(tmp, m_col, s)
    nc.vector.tensor_sub(t, b_col, tmp)
    return s, t


def _emit(nc, tc, t_in, t_w, t_out, dbg):
    with ExitStack() as ctx:
        const = ctx.enter_context(tc.tile_pool(name="const", bufs=1))
        pers = ctx.enter_context(tc.tile_pool(name="pers", bufs=1))
        ps_s = ctx.enter_context(tc.tile_pool(name="ps_s", bufs=3, space="PSUM"))
        ps_e = ctx.enter_context(tc.tile_pool(name="ps_e", bufs=2, space="PSUM"))
        ps_m = ctx.enter_context(tc.tile_pool(name="ps_m", bufs=2, space="PSUM"))
        mstack = ExitStack()  # closed before the head to free SBUF
        sbs = mstack.enter_context(tc.tile_pool(name="sbs", bufs=2))   # s_sb
        sbw = mstack.enter_context(tc.tile_pool(name="sbw", bufs=2))   # small work tiles
        sbg = mstack.enter_context(tc.tile_pool(name="sbg", bufs=2))   # gather out
        sbx = mstack.enter_context(tc.tile_pool(name="sbx", bufs=1))   # x slots (tagged)

        ident = const.tile([P, P], F32)
        make_identity(nc, ident[:])
        ident16 = const.tile([P, P], F16)
        nc.vector.tensor_copy(out=ident16, in_=ident)
        ones_row = const.tile([1, P], F32)
        nc.vector.memset(ones_row, 1.0)
        ones_col = const.tile([P, 1], F32)
        nc.vector.memset(ones_col, 1.0)
        eps_col = const.tile([P, 1], F32)
        nc.vector.memset(eps_col, EPS)

        # SELR[g][p, p'] = 1 iff p == g*16 + p' % 16  (wrapped-idx builder), fp16
        selr = const.tile([P, 8, P], F16)
        for g in range(8):
            isrc = ident16[:, g * 16:(g + 1) * 16]
            src_b = bass.AP(tensor=isrc.tensor, offset=isrc.offset,
                            ap=[isrc.ap[0], [0, 8], isrc.ap[1]])
            nc.vector.tensor_copy(
                out=selr[:, g, :].rearrange("p (o q) -> p o q", q=16), in_=src_b)

        # persistent f32 layer outputs (conv5 cat operands + next-layer inputs)
        xp = [pers.tile([65, N], F32, name="x0p"),
              pers.tile([65, N], F32, name="x1p"),
              pers.tile([P, N], F32, name="x2p"),
              pers.tile([P, 2 * N], F32, name="x3p")]
        p_cf = pers.tile([P, 16], F32)
        mean_z = pers.tile([P, 8, NG], F32)    # sum of pre-act h per (j, group)
        mean_r = pers.tile([P, 8, NG], F32)    # sum of relu(-h)

        # conv5 weights: W4T chains [crow, 1024] fp32
        chains = [(0, 64, 0), (1, 64, 0), (2, 128, 0), (3, 128, 0), (3, 128, N)]
        # (source xp idx, rows, free offset); W4T row offsets:
        c4off = [0, 64, 128, 256, 384]
        w4c = [pers.tile([crow, 1024], F32, name=f"w4c{ci}")
               for ci, (_, crow, _) in enumerate(chains)]
        for ci, (_, crow, _) in enumerate(chains):
            nc.sync.dma_start(out=w4c[ci], in_=t_w["w4t"][c4off[ci]:c4off[ci] + crow, :])
        s4 = pers.tile([P, 8], F32)
        t4 = pers.tile([P, 8], F32)
        for j in range(8):
            nc.sync.dma_start(out=s4[:, j:j + 1], in_=t_w["bns4"][j * P:(j + 1) * P, :])
            nc.sync.dma_start(out=t4[:, j:j + 1], in_=t_w["bnt4"][j * P:(j + 1) * P, :])

        # ---------------- input transpose: feat [N, 3] -> xB0 [3, N], xA0 = 2x --
        xa0 = sbx.tile([P, N], F32, tag="xA0", name="xA0")
        xb0 = sbx.tile([5, N], F32, tag="xB0", name="xB0")
        nsq0 = xb0[3:4, :]
        for t in range(NT):
            ft = sbw.tile([P, 3], F32, tag="feat")
            nc.sync.dma_start(out=ft, in_=t_in["feat_xyz"][t * P:(t + 1) * P, :])
            pt = ps_m.tile([P, P], F32, tag="m")
            nc.tensor.transpose(out=pt[0:3, 0:P], in_=ft[:, :], identity=ident)
            sl = slice(t * P, (t + 1) * P)
            nc.scalar.activation(out=xb0[0:3, sl], in_=pt[0:3, 0:P], func=AF.Copy)
            nc.scalar.activation(out=xa0[0:3, sl], in_=pt[0:3, 0:P], func=AF.Copy,
                                 scale=2.0)
        nc.sync.dma_start(out=xa0[3:4, :], in_=t_w["onesN"][:, :])

        xa, xb, nsq = xa0, xb0, nsq0
        # =================== edge conv layers ===================
        for li, (O, C) in enumerate(CONV):
            OCH = (O + P - 1) // P
            is3 = (C == P)
            with ExitStack() as lctx:
                sb = lctx.enter_context(tc.tile_pool(name=f"sb_l{li}", bufs=1))
                u_dram = t_w[f"Utab{li}"]

                # --- weights (host-pretransposed)
                wlT = sb.tile([P, O], F32, tag="wlT")
                wvT = sb.tile([P, O], F32, tag="wvT")
                nc.sync.dma_start(out=wlT[0:C, :], in_=t_w[f"wlT{li}"][:, :])
                nc.sync.dma_start(out=wvT[0:C, :], in_=t_w[f"wvT{li}"][:, :])
                bns = sb.tile([P, OCH], F32, tag="bns")
                bnt = sb.tile([P, OCH], F32, tag="bnt")
                for j in range(OCH):
                    ow = min(P, O - j * P)
                    nc.sync.dma_start(out=bns[0:ow, j:j + 1],
                                      in_=t_w[f"bns{li}"][j * P:j * P + ow, :])
                    nc.sync.dma_start(out=bnt[0:ow, j:j + 1],
                                      in_=t_w[f"bnt{li}"][j * P:j * P + ow, :])

                # --- nsq row: -sum_c x^2 (scratch at partition 0, DMA to row C)
                for q in range(4):
                    sl = slice(q * 512, (q + 1) * 512)
                    xxb = sbw.tile([P, 512], F32, tag="xx")
                    nc.scalar.activation(out=xxb[0:C, :], in_=xb[0:C, sl],
                                         func=AF.Square)
                    pq = ps_m.tile([1, 512], F32, tag="m")
                    nc.tensor.matmul(out=pq, lhsT=ones_col[0:C, :], rhs=xxb[0:C, :],
                                     start=True, stop=True)
                    nscr = sbw.tile([1, 512], F32, tag="nsq_scr")
                    nc.scalar.activation(out=nscr, in_=pq, func=AF.Copy, scale=-1.0)
                    nc.sync.dma_start(out=nsq[:, sl], in_=nscr)

                # --- U table -> DRAM
                for t in range(NT):
                    pu = ps_m.tile([P, 512], F32, tag="m")
                    nc.tensor.matmul(out=pu[:, 0:O], lhsT=xb[0:C, t * P:(t + 1) * P],
                                     rhs=wlT[0:C, 0:O], start=True, stop=True)
                    usb = sbw.tile([P, O], F32, tag="u_sb")
                    nc.scalar.activation(out=usb, in_=pu[:, 0:O], func=AF.Copy)
                    nc.sync.dma_start(out=u_dram[t * P:(t + 1) * P, :], in_=usb)

                # next-layer xA slot (xB comes from persistent xp[li])
                if li < 3:
                    nxa = sbx.tile([P, N], F32, tag=f"xA{(li + 1) % 2}",
                                   name=f"xA{li + 1}")
                else:
                    nxa = None
                def epilogue(g, m_grp):
                    """conv epilogue for group g (points g*512:(g+1)*512)."""
                    gsl = slice(g * 512, (g + 1) * 512)
                    for j in range(OCH):
                        ow = min(P, O - j * P)
                        pe = ps_e.tile([P, 512], F32, tag="e_ps")
                        nc.tensor.matmul(out=pe[0:ow, :],
                                         lhsT=wvT[0:C, j * P:j * P + ow],
                                         rhs=xb[0:C, gsl], start=True, stop=False)
                        for tt in range(4):
                            msl = m_grp[:, tt * O + j * P: tt * O + j * P + ow]
                            nc.tensor.matmul(
                                out=pe[0:ow, tt * P:(tt + 1) * P],
                                lhsT=msl, rhs=ident,
                                is_transpose=True, start=False, stop=(tt == 3),
                                skip_group_check=True)
                        # y = lrelu(bn(...)); write to next-layer xB (or scratch for l3)
                        if li < 3:
                            # OCH == 1 always here (O <= 128), so j == 0
                            dst = xp[li][j * P:j * P + ow, gsl]
                        else:
                            dst = xp[3][:, j * N + g * 512:j * N + (g + 1) * 512]
                        nc.scalar.activation(out=dst, in_=pe[0:ow, :],
                                             func=AF.Identity,
                                             scale=bns[0:ow, j:j + 1],
                                             bias=bnt[0:ow, j:j + 1])
                        tmp = sbw.tile([P, 512], F32, tag="lr_tmp")
                        nc.vector.tensor_scalar_mul(tmp[0:ow, :], dst, LRELU_CONV)
                        nc.vector.tensor_tensor(out=dst, in0=dst,
                                                in1=tmp[0:ow, :], op=ALU.max)
                        if li < 3:
                            nc.scalar.activation(out=nxa[j * P:j * P + ow, gsl],
                                                 in_=dst, func=AF.Copy, scale=2.0)

                def conv5(g):
                    """1024-ch conv + pooling for group g (after layer-3 epilogue)."""
                    gsl = slice(g * 512, (g + 1) * 512)
                    for j in range(8):
                        pc = ps_e.tile([P, 512], F32, tag="e_ps")
                        for ci, (lx, crow, fo) in enumerate(chains):
                            fsl = slice(fo + g * 512, fo + (g + 1) * 512)
                            nc.tensor.matmul(
                                out=pc, lhsT=w4c[ci][0:crow, j * P:(j + 1) * P],
                                rhs=xp[lx][0:crow, fsl],
                                start=(ci == 0), stop=(ci == len(chains) - 1))
                        # h pre-act; mean accumulators via two Act passes
                        hs = sbw.tile([P, 512], F32, tag="h_sb")
                        nc.scalar.activation(out=hs, in_=pc, func=AF.Identity,
                                             scale=s4[:, j:j + 1], bias=t4[:, j:j + 1],
                                             accum_out=mean_z[:, j, g:g + 1])
                        hr = sbw.tile([P, 512], F32, tag="lr_tmp")
                        nc.scalar.activation(out=hr, in_=hs, func=AF.Relu, scale=-1.0,
                                             accum_out=mean_r[:, j, g:g + 1])
                        # max-pool on pre-act h (lrelu applied to pooled value later)
                        pm = sbw.tile([P, 1], F32, tag="pmax")
                        nc.vector.tensor_reduce(out=pm, in_=hs, axis=AX.X, op=ALU.max)
                        if g == 0:
                            nc.vector.tensor_copy(out=p_cf[:, j:j + 1], in_=pm)
                        else:
                            nc.vector.tensor_tensor(out=p_cf[:, j:j + 1],
                                                    in0=p_cf[:, j:j + 1], in1=pm,
                                                    op=ALU.max)

                # --- per point-tile: S, top-k, idx wrap, gather, k-reduce
                # S of tile t+1 is emitted before tile t's top-k so the PE
                # computes it during the DVE scans (instead of idling behind
                # the selr matmuls that wait on the top-k).
                def emit_S(t):
                    s_sb = sbs.tile([P, N], F32, tag="s_sb", name="s_sb")
                    for q in range(4):
                        sl = slice(q * 512, (q + 1) * 512)
                        pq = ps_s.tile([P, 512], F32, tag="s_ps")
                        if not is3:
                            nc.tensor.matmul(out=pq,
                                             lhsT=xa[0:C + 1, t * P:(t + 1) * P],
                                             rhs=xb[0:C + 1, sl],
                                             start=True, stop=True)
                        else:
                            nc.tensor.matmul(out=pq,
                                             lhsT=xa[0:C, t * P:(t + 1) * P],
                                             rhs=xb[0:C, sl], start=True, stop=False)
                            nc.tensor.matmul(out=pq, lhsT=ones_row, rhs=nsq[:, sl],
                                             start=False, stop=True)
                        nc.scalar.activation(out=s_sb[:, sl], in_=pq, func=AF.Copy)
                    return s_sb

                pending = []
                epiq = []
                cur_m = [None]
                s_cur = emit_S(0)
                for t in range(NT):
                    if t % 4 == 0:
                        cur_m[0] = sbs.tile([P, 4 * O], F32, tag="m_grp", name="m_grp")
                    s_nxt = emit_S(t + 1) if t + 1 < NT else None
                    s_sb = s_cur
                    v24 = sbw.tile([P, 24], F32, tag="v24")
                    i24 = sbw.tile([P, 24], U32, tag="i24")
                    nc.vector.max(out=v24[:, 0:8], in_=s_sb)
                    nc.vector.max_index(out=i24[:, 0:8], in_max=v24[:, 0:8],
                                        in_values=s_sb)
                    nc.vector.match_replace(out=s_sb, in_to_replace=v24[:, 0:8],
                                            in_values=s_sb, imm_value=NEG)
                    nc.vector.max(out=v24[:, 8:16], in_=s_sb)
                    nc.vector.max_index(out=i24[:, 8:16], in_max=v24[:, 8:16],
                                        in_values=s_sb)
                    nc.vector.match_replace(out=s_sb, in_to_replace=v24[:, 8:16],
                                            in_values=s_sb, imm_value=NEG)
                    nc.vector.max(out=v24[:, 16:24], in_=s_sb)
                    nc.vector.max_index(out=i24[:, 16:24], in_max=v24[:, 16:24],
                                        in_values=s_sb)

                    idxf32 = sbw.tile([P, KNN], F32, tag="idxf32")
                    nc.vector.tensor_copy(out=idxf32, in_=i24[:, 0:KNN])
                    idxf = sbw.tile([P, KNN], F16, tag="idxf")
                    nc.vector.tensor_copy(out=idxf, in_=idxf32)
                    pw = ps_m.tile([P, 8 * KNN], F32, tag="m")
                    for g8 in range(8):
                        nc.tensor.matmul(
                            out=pw[:, :].rearrange("p (k g) -> p k g", g=8)[:, :, g8],
                            lhsT=selr[:, g8, :], rhs=idxf, start=True, stop=True,
                            skip_group_check=True)
                    w16 = sbw.tile([P, 8 * KNN], I16, tag="w16")
                    nc.vector.tensor_copy(out=w16, in_=pw)

                    # two half-gathers (neighbors 0-9 / 10-19): the k-reduce of
                    # half A can overlap the gather of half B, halving the
                    # window where the DVE waits on the (Q7-bound) gather.
                    KH = KNN // 2
                    gta = sbg.tile([P, KH, O], F32, tag="gather", name="gta")
                    nc.gpsimd.dma_gather(
                        out_ap=gta[:, :, :], in_ap=u_dram[:, :],
                        idxs_ap=w16[:, 0:KH * 8], num_idxs=P * KH,
                        num_idxs_reg=P * KH, elem_size=O, single_packet=False)
                    gtb = sbg.tile([P, KH, O], F32, tag="gather", name="gtb")
                    nc.gpsimd.dma_gather(
                        out_ap=gtb[:, :, :], in_ap=u_dram[:, :],
                        idxs_ap=w16[:, KH * 8:KNN * 8], num_idxs=P * KH,
                        num_idxs_reg=P * KH, elem_size=O, single_packet=False)
                    pending.append((t, gta, gtb, cur_m[0]))

                    def flush_one():
                        tk, gak, gbk, mgk = pending.pop(0)
                        msl = mgk[:, (tk % 4) * O:(tk % 4 + 1) * O]
                        nc.vector.tensor_reduce(
                            out=msl,
                            in_=gak[:, :, :].rearrange("p k o -> p o k"),
                            axis=AX.X, op=ALU.max)
                        mtmp = sbw.tile([P, O], F32, tag="mtmp")
                        nc.vector.tensor_reduce(
                            out=mtmp,
                            in_=gbk[:, :, :].rearrange("p k o -> p o k"),
                            axis=AX.X, op=ALU.max)
                        nc.vector.tensor_tensor(out=msl, in0=msl, in1=mtmp,
                                                op=ALU.max)
                        if tk % 4 == 3:
                            epiq.append((tk // 4, mgk))

                    def drain_epis(now):
                        while epiq and (now or epiq[0][0] * 4 + 6 <= t):
                            g, mgk = epiq.pop(0)
                            epilogue(g, mgk)
                            if li == 3:
                                conv5(g)

                    # lag-1 software pipeline: k-reduce of tile t-1 issues after
                    # tile t's top-k, so the DVE never stalls on the gather.
                    # Epilogues are emitted 2+ tiles later still, so their PE ops
                    # never block upcoming S matmuls on not-yet-passed DVE points.
                    if len(pending) > 1:
                        flush_one()
                    drain_epis(False)
                    if t == NT - 1:
                        while pending:
                            flush_one()
                        drain_epis(True)
                    s_cur = s_nxt
                if li < 3:
                    # ones row of next xA (if next layer has aug row)
                    if CONV[li + 1][1] < P:
                        nc.sync.dma_start(
                            out=nxa[CONV[li + 1][1]:CONV[li + 1][1] + 1, :],
                            in_=t_w["onesN"][:, :])
                if dbg:
                    nc.sync.dma_start(out=t_out[f"dbg_x{li}"][:, :],
                                      in_=xp[li][0:min(O, P), :])
            if li < 3:
                xa = nxa
                xb = xp[li]
                if CONV[li + 1][1] < P:
                    nsq = xp[li][CONV[li + 1][1]:CONV[li + 1][1] + 1, :]
                else:
                    nsq = sbx.tile([1, N], F32, tag="nsq3", name="nsq3")

        # =================== finish pooling ===================
        # mean = (sum_z - 0.8 * sum_relu(-z)) / N ; p_cf[:, 8+j]
        with tc.tile_pool(name="sb_pool", bufs=1) as sbp:
            mz = sbp.tile([P, 8], F32)
            mr = sbp.tile([P, 8], F32)
            nc.vector.tensor_reduce(out=mz, in_=mean_z[:, :, :], axis=AX.X, op=ALU.add)
            nc.vector.tensor_reduce(out=mr, in_=mean_r[:, :, :], axis=AX.X, op=ALU.add)
            # sum lrelu(z) = sum z + (1 - alpha) * sum relu(-z)
            nc.vector.tensor_scalar_mul(mr, mr, 1.0 - LRELU_CONV)
            nc.vector.tensor_add(out=p_cf[:, 8:16], in0=mz, in1=mr)
            nc.vector.tensor_scalar_mul(p_cf[:, 8:16], p_cf[:, 8:16], 1.0 / N)
            # lrelu on max-pooled columns (monotone: lrelu(max) = max(lrelu))
            t8 = sbp.tile([P, 8], F32)
            nc.vector.tensor_scalar_mul(t8, p_cf[:, 0:8], LRELU_CONV)
            nc.vector.tensor_tensor(out=p_cf[:, 0:8], in0=p_cf[:, 0:8], in1=t8,
                                    op=ALU.max)
            if dbg:
                nc.sync.dma_start(out=t_out["dbg_p"][:, :], in_=p_cf[:, :])

        mstack.close()
        # =================== MLP head (broadcast + DVE dot-products) ==========
        with ExitStack() as hctx:
            sb = hctx.enter_context(tc.tile_pool(name="sb_head", bufs=1))
            sbwh = hctx.enter_context(tc.tile_pool(name="sbw_head", bufs=2))

            def lin(name, src_col, incols, w_dram, out_dim, alpha):
                in_dim = P * incols
                och = (out_dim + P - 1) // P
                orows = min(P, out_dim)
                bcast = sb.tile([P, in_dim], F32, tag=f"{name}_bc")
                for j in range(incols):
                    pT = ps_m.tile([1, P], F32, tag="m")
                    nc.tensor.transpose(out=pT, in_=src_col[:, j:j + 1],
                                        identity=ident)
                    rowj = sbwh.tile([1, P], F32, tag="hd_row")
                    nc.scalar.activation(out=rowj, in_=pT, func=AF.Copy)
                    pb = ps_m.tile([P, P], F32, tag="m")
                    nc.tensor.matmul(out=pb, lhsT=ones_row, rhs=rowj,
                                     start=True, stop=True)
                    nc.scalar.activation(out=bcast[:, j * P:(j + 1) * P], in_=pb,
                                         func=AF.Copy)
                dst = sb.tile([P, och], F32, tag=f"{name}_out")
                for ot in range(och):
                    orw = min(P, out_dim - ot * P)
                    wsb = sbwh.tile([P, in_dim], F32, tag=f"{name}_w")
                    nc.sync.dma_start(out=wsb[0:orw, :],
                                      in_=w_dram[ot * P:ot * P + orw, :])
                    prod = sbwh.tile([P, in_dim], F32, tag=f"{name}_prod")
                    nc.vector.tensor_mul(prod[0:orw, :], wsb[0:orw, :], bcast[0:orw, :])
                    nc.vector.tensor_reduce(out=dst[0:orw, ot:ot + 1],
                                            in_=prod[0:orw, :], axis=AX.X, op=ALU.add)
                if alpha is not None:
                    tmp = sbwh.tile([P, och], F32, tag=f"{name}_tmp")
                    nc.vector.tensor_scalar_mul(tmp[0:orows, :], dst[0:orows, :], alpha)
                    nc.vector.tensor_tensor(out=dst[0:orows, :], in0=dst[0:orows, :],
                                            in1=tmp[0:orows, :], op=ALU.max)
                return dst

            y1 = lin("y1", p_cf, 16, t_w["L1"], 512, LRELU_HEAD)
            y2 = lin("y2", y1, 4, t_w["L2"], 256, LRELU_HEAD)
            y3 = lin("y3", y2, 2, t_w["L3"], 128, LRELU_HEAD)
            y4 = lin("y4", y3, 1, t_w["L4"], 2, None)
            osb = sb.tile([2, 1], F32, tag="out_sb")
            nc.vector.tensor_copy(out=osb, in_=y4[0:2, 0:1])
            nc.sync.dma_start(out=t_out["out"][:, :], in_=osb)


_PROG_CACHE = {}


def _build(dbg=False):
    key = ("v2", dbg)
    if key in _PROG_CACHE:
        return _PROG_CACHE[key]
    nc = bacc.Bacc("TRN2", target_bir_lowering=False, debug=False, num_devices=B)
    t_in = {"feat_xyz": nc.declare_dram_parameter("feat_xyz", [N, 3], F32,
                                                  isOutput=False)}
    t_w = {}
    for li, (O, C) in enumerate(CONV):
        t_w[f"wlT{li}"] = nc.declare_dram_parameter(f"wlT{li}", [C, O], F32,
                                                    isOutput=False)
        t_w[f"wvT{li}"] = nc.declare_dram_parameter(f"wvT{li}", [C, O], F32,
                                                    isOutput=False)
        t_w[f"bns{li}"] = nc.declare_dram_parameter(f"bns{li}", [O, 1], F32,
                                                    isOutput=False)
        t_w[f"bnt{li}"] = nc.declare_dram_parameter(f"bnt{li}", [O, 1], F32,
                                                    isOutput=False)
        t_w[f"Utab{li}"] = nc.dram_tensor(f"Utab{li}", [N, O], F32)
    t_w["w4t"] = nc.declare_dram_parameter("w4t", [512, 1024], F32,
                                           isOutput=False)
    t_w["onesN"] = nc.declare_dram_parameter("onesN", [1, N], F32, isOutput=False)
    t_w["bns4"] = nc.declare_dram_parameter("bns4", [1024, 1], F32, isOutput=False)
    t_w["bnt4"] = nc.declare_dram_parameter("bnt4", [1024, 1], F32, isOutput=False)
    for j, (o, c) in enumerate(LIN):
        t_w[f"L{j+1}"] = nc.declare_dram_parameter(f"L{j+1}", [o, c], F32,
                                                   isOutput=False)
    t_out = {"out": nc.declare_dram_parameter("out", [2, 1], F32, isOutput=True)}
    if dbg:
        for li in range(4):
            O = CONV[li][0]
            sh = [P, 2 * N] if O == 256 else [O, N]
            t_out[f"dbg_x{li}"] = nc.declare_dram_parameter(f"dbg_x{li}", sh,
                                                            F32, isOutput=True)
        t_out["dbg_p"] = nc.declare_dram_parameter("dbg_p", [P, 16], F32,
                                                   isOutput=True)

    with tile.TileContext(nc) as tc:
        _emit(nc, tc, t_in, t_w, t_out, dbg)
    nc.compile()
    _PROG_CACHE[key] = nc
    return nc


def _make_in_maps(inputs):
    f32 = lambda a: np.ascontiguousarray(np.asarray(a, np.float32))
    feat = f32(inputs["feat_xyz"])
    common = {}
    for li, (O, C) in enumerate(CONV):
        W = f32(inputs[f"W{li}"])
        wl, wr = W[:, :C], W[:, C:]
        common[f"wlT{li}"] = f32(wl.T)
        common[f"wvT{li}"] = f32((wr - wl).T)
        g, b, m, v = (f32(inputs[f"{n}{li}"]) for n in "gbmv")
        s = g / np.sqrt(v + EPS)
        common[f"bns{li}"] = f32(s.reshape(-1, 1))
        common[f"bnt{li}"] = f32((b - m * s).reshape(-1, 1))
    common["w4t"] = np.ascontiguousarray(f32(inputs["W4"]).T)   # [512, 1024]
    g, b, m, v = (f32(inputs[f"{n}4"]) for n in "gbmv")
    s = g / np.sqrt(v + EPS)
    common["bns4"] = f32(s.reshape(-1, 1))
    common["bnt4"] = f32((b - m * s).reshape(-1, 1))
    common["onesN"] = np.ones((1, N), np.float32)
    for j in range(1, 5):
        common[f"L{j}"] = f32(inputs[f"L{j}"])
    return [dict(common, feat_xyz=np.ascontiguousarray(feat[b])) for b in range(B)]


def run(inputs, dbg=False, trace=False, **kw):
    nc = _build(dbg)
    in_maps = _make_in_maps(inputs)
    return run_bass_kernel_spmd(nc, in_maps, list(range(B)), trace=trace, **kw)


def kernel(**inputs):
    res = run(inputs).results
    out = np.stack([res[b]["out"][:, 0] for b in range(B)], axis=0)
    return out.astype(np.float32)


# revision 22
# speedup vs baseline: 1.3459x; 1.2160x over previous
"""DGCNN forward kernel for Trainium2 (8 NeuronCores, data-parallel over batch).

Each core processes one point cloud (N=2048 points) end to end:
  4x EdgeConv (KNN k=20 + 1x1 conv + BN + LeakyReLU(0.2) + max over k)
  -> concat -> 1x1 conv to 1024 + BN + LeakyReLU -> global max+mean pool
  -> MLP 2048-512-256-128-2 with LeakyReLU(0.01).

Algebraic rewrite (as baseline): max_k f(W @ [nbr - ctr, ctr]) = f(max_k(U[idx_k]) + V)
with U = Wl @ x, V = (Wr - Wl) @ x.

v2 changes vs baseline (same fp32 trunk numerics, better engine balance):
  - nsq folded into the S matmul as an extra contraction row (lhsT gets a ones
    row, rhs gets the -|x|^2 row) -> halves fp32 S-matmul column streams.
  - xA holds 2*x so the distance matmul needs no separate doubling.
  - index-wrap (selr) matmuls in fp16 (indices < 2048 are exact in fp16).
  - weights pre-transposed/pre-folded on host (wlT/wvT/bn s,t/W4T hi-lo).
  - conv5 in split-bf16 (3 terms) riding under the layer-3 pipeline; pooling
    via monotone max (pre-activation) + Act accumulators for the mean.
  - per-4-tile-group epilogues (fatter matmuls/activations).
"""

import numpy as np
from contextlib import ExitStack

import concourse.bass as bass
import concourse.bacc as bacc
import concourse.tile as tile
from concourse import mybir
from concourse.bass_utils import run_bass_kernel_spmd
from concourse.masks import make_identity
from concourse import hw_specs as _hw_specs

# The stock cost model assumes 0.34ns/descriptor for SWDGE (software DGE)
# descriptor generation; measured hardware cost for dma_gather is ~8ns/desc
# (20.4us per 2560-index gather). With the stock value the tile scheduler
# believes gathers are ~2us and schedules their consumers (k-reduce,
# epilogue matmuls) immediately behind them, stalling every engine queue for
# ~20us per tile. Correcting the constant lets the scheduler overlap the
# gathers with independent work.
_hw_specs.TRN2Spec.SWDGE_NS_PER_DESCRIPTOR = 8.0

F32 = mybir.dt.float32
F16 = mybir.dt.float16
BF16 = mybir.dt.bfloat16
I16 = mybir.dt.int16
U32 = mybir.dt.uint32
AF = mybir.ActivationFunctionType
ALU = mybir.AluOpType
AX = mybir.AxisListType

B, N, KNN, P = 8, 2048, 20, 128
NT = N // P                      # 16 point tiles
NG = NT // 4                     # 4 groups of 4 tiles
EPS = 1e-5
NEG = -1e30
CONV = [(64, 3), (64, 64), (128, 64), (256, 128)]   # (O, C) of edge convs
LIN = [(512, 2048), (256, 512), (128, 256), (2, 128)]
LRELU_CONV = 0.2
LRELU_HEAD = 0.01


def _bn_fold(nc, sb, g_col, b_col, m_col, v_col, ncols, eps_col):
    """s = g * rsqrt(v + eps); t = b - m * s  (all [128, ncols] column tiles)."""
    s = sb.tile([P, ncols], F32, tag="bn_s")
    t = sb.tile([P, ncols], F32, tag="bn_t")
    tmp = sb.tile([P, ncols], F32, tag="bn_tmp")
    nc.scalar.activation(out=tmp, in_=v_col, func=AF.Sqrt, bias=eps_col, scale=1.0)
    nc.vector.reciprocal(out=s, in_=tmp)
    nc.vector.tensor_mul(s, s, g_col)
    nc.vector.tensor_mul(tmp, m_col, s)
    nc.vector.tensor_sub(t, b_col, tmp)
    return s, t


def _emit(nc, tc, t_in, t_w, t_out, dbg):
    with ExitStack() as ctx:
        const = ctx.enter_context(tc.tile_pool(name="const", bufs=1))
        pers = ctx.enter_context(tc.tile_pool(name="pers", bufs=1))
        ps_s = ctx.enter_context(tc.tile_pool(name="ps_s", bufs=3, space="PSUM"))
        ps_e = ctx.enter_context(tc.tile_pool(name="ps_e", bufs=2, space="PSUM"))
        ps_m = ctx.enter_context(tc.tile_pool(name="ps_m", bufs=2, space="PSUM"))
        mstack = ExitStack()  # closed before the head to free SBUF
        sbs = mstack.enter_context(tc.tile_pool(name="sbs", bufs=2))   # s_sb
        sbw = mstack.enter_context(tc.tile_pool(name="sbw", bufs=2))   # small work tiles
        sbg = mstack.enter_context(tc.tile_pool(name="sbg", bufs=2))   # gather out
        sbx = mstack.enter_context(tc.tile_pool(name="sbx", bufs=1))   # x slots (tagged)

        ident = const.tile([P, P], F32)
        make_identity(nc, ident[:])
        ident16 = const.tile([P, P], F16)
        nc.vector.tensor_copy(out=ident16, in_=ident)
        ones_row = const.tile([1, P], F32)
        nc.vector.memset(ones_row, 1.0)
        ones_col = const.tile([P, 1], F32)
        nc.vector.memset(ones_col, 1.0)
        eps_col = const.tile([P, 1], F32)
        nc.vector.memset(eps_col, EPS)

        # SELR[g][p, p'] = 1 iff p == g*16 + p' % 16  (wrapped-idx builder), fp16
        selr = const.tile([P, 8, P], F16)
        for g in range(8):
            isrc = ident16[:, g * 16:(g + 1) * 16]
            src_b = bass.AP(tensor=isrc.tensor, offset=isrc.offset,
                            ap=[isrc.ap[0], [0, 8], isrc.ap[1]])
            nc.vector.tensor_copy(
                out=selr[:, g, :].rearrange("p (o q) -> p o q", q=16), in_=src_b)

        # persistent f32 layer outputs (conv5 cat operands + next-layer inputs)
        xp = [pers.tile([65, N], F32, name="x0p"),
              pers.tile([65, N], F32, name="x1p"),
              pers.tile([P, N], F32, name="x2p"),
              pers.tile([P, 2 * N], F32, name="x3p")]
        p_cf = pers.tile([P, 16], F32)
        mean_z = pers.tile([P, 8, NG], F32)    # sum of pre-act h per (j, group)
        mean_r = pers.tile([P, 8, NG], F32)    # sum of relu(-h)

        # conv5 weights: W4T chains [crow, 1024] fp32
        chains = [(0, 64, 0), (1, 64, 0), (2, 128, 0), (3, 128, 0), (3, 128, N)]
        # (source xp idx, rows, free offset); W4T row offsets:
        c4off = [0, 64, 128, 256, 384]
        w4c = [pers.tile([crow, 1024], F32, name=f"w4c{ci}")
               for ci, (_, crow, _) in enumerate(chains)]
        for ci, (_, crow, _) in enumerate(chains):
            nc.sync.dma_start(out=w4c[ci], in_=t_w["w4t"][c4off[ci]:c4off[ci] + crow, :])
        s4 = pers.tile([P, 8], F32)
        t4 = pers.tile([P, 8], F32)
        for j in range(8):
            nc.sync.dma_start(out=s4[:, j:j + 1], in_=t_w["bns4"][j * P:(j + 1) * P, :])
            nc.sync.dma_start(out=t4[:, j:j + 1], in_=t_w["bnt4"][j * P:(j + 1) * P, :])

        # ---------------- input transpose: feat [N, 3] -> xB0 [3, N], xA0 = 2x --
        xa0 = sbx.tile([P, N], F32, tag="xA0", name="xA0")
        xb0 = sbx.tile([5, N], F32, tag="xB0", name="xB0")
        nsq0 = xb0[3:4, :]
        for t in range(NT):
            ft = sbw.tile([P, 3], F32, tag="feat")
            nc.sync.dma_start(out=ft, in_=t_in["feat_xyz"][t * P:(t + 1) * P, :])
            pt = ps_m.tile([P, P], F32, tag="m")
            nc.tensor.transpose(out=pt[0:3, 0:P], in_=ft[:, :], identity=ident)
            sl = slice(t * P, (t + 1) * P)
            nc.scalar.activation(out=xb0[0:3, sl], in_=pt[0:3, 0:P], func=AF.Copy)
            nc.scalar.activation(out=xa0[0:3, sl], in_=pt[0:3, 0:P], func=AF.Copy,
                                 scale=2.0)
        nc.sync.dma_start(out=xa0[3:4, :], in_=t_w["onesN"][:, :])

        xa, xb, nsq = xa0, xb0, nsq0
        # =================== edge conv layers ===================
        for li, (O, C) in enumerate(CONV):
            OCH = (O + P - 1) // P
            is3 = (C == P)
            with ExitStack() as lctx:
                sb = lctx.enter_context(tc.tile_pool(name=f"sb_l{li}", bufs=1))
                u_dram = t_w[f"Utab{li}"]

                # --- weights (host-pretransposed)
                wlT = sb.tile([P, O], F32, tag="wlT")
                wvT = sb.tile([P, O], F32, tag="wvT")
                nc.sync.dma_start(out=wlT[0:C, :], in_=t_w[f"wlT{li}"][:, :])
                nc.sync.dma_start(out=wvT[0:C, :], in_=t_w[f"wvT{li}"][:, :])
                bns = sb.tile([P, OCH], F32, tag="bns")
                bnt = sb.tile([P, OCH], F32, tag="bnt")
                for j in range(OCH):
                    ow = min(P, O - j * P)
                    nc.sync.dma_start(out=bns[0:ow, j:j + 1],
                                      in_=t_w[f"bns{li}"][j * P:j * P + ow, :])
                    nc.sync.dma_start(out=bnt[0:ow, j:j + 1],
                                      in_=t_w[f"bnt{li}"][j * P:j * P + ow, :])

                # --- nsq row: -sum_c x^2 (scratch at partition 0, DMA to row C)
                for q in range(4):
                    sl = slice(q * 512, (q + 1) * 512)
                    xxb = sbw.tile([P, 512], F32, tag="xx")
                    nc.scalar.activation(out=xxb[0:C, :], in_=xb[0:C, sl],
                                         func=AF.Square)
                    pq = ps_m.tile([1, 512], F32, tag="m")
                    nc.tensor.matmul(out=pq, lhsT=ones_col[0:C, :], rhs=xxb[0:C, :],
                                     start=True, stop=True)
                    nscr = sbw.tile([1, 512], F32, tag="nsq_scr")
                    nc.scalar.activation(out=nscr, in_=pq, func=AF.Copy, scale=-1.0)
                    nc.sync.dma_start(out=nsq[:, sl], in_=nscr)

                # --- U table -> DRAM
                for t in range(NT):
                    pu = ps_m.tile([P, 512], F32, tag="m")
                    nc.tensor.matmul(out=pu[:, 0:O], lhsT=xb[0:C, t * P:(t + 1) * P],
                                     rhs=wlT[0:C, 0:O], start=True, stop=True)
                    usb = sbw.tile([P, O], F32, tag="u_sb")
                    nc.scalar.activation(out=usb, in_=pu[:, 0:O], func=AF.Copy)
                    nc.sync.dma_start(out=u_dram[t * P:(t + 1) * P, :], in_=usb)

                # next-layer xA slot (xB comes from persistent xp[li])
                if li < 3:
                    nxa = sbx.tile([P, N], F32, tag=f"xA{(li + 1) % 2}",
                                   name=f"xA{li + 1}")
                else:
                    nxa = None
                def epilogue(g, m_grp):
                    """conv epilogue for group g (points g*512:(g+1)*512)."""
                    gsl = slice(g * 512, (g + 1) * 512)
                    for j in range(OCH):
                        ow = min(P, O - j * P)
                        pe = ps_e.tile([P, 512], F32, tag="e_ps")
                        nc.tensor.matmul(out=pe[0:ow, :],
                                         lhsT=wvT[0:C, j * P:j * P + ow],
                                         rhs=xb[0:C, gsl], start=True, stop=False)
                        for tt in range(4):
                            msl = m_grp[:, tt * O + j * P: tt * O + j * P + ow]
                            nc.tensor.matmul(
                                out=pe[0:ow, tt * P:(tt + 1) * P],
                                lhsT=msl, rhs=ident,
                                is_transpose=True, start=False, stop=(tt == 3),
                                skip_group_check=True)
                        # y = lrelu(bn(...)); write to next-layer xB (or scratch for l3)
                        if li < 3:
                            # OCH == 1 always here (O <= 128), so j == 0
                            dst = xp[li][j * P:j * P + ow, gsl]
                        else:
                            dst = xp[3][:, j * N + g * 512:j * N + (g + 1) * 512]
                        nc.scalar.activation(out=dst, in_=pe[0:ow, :],
                                             func=AF.Identity,
                                             scale=bns[0:ow, j:j + 1],
                                             bias=bnt[0:ow, j:j + 1])
                        tmp = sbw.tile([P, 512], F32, tag="lr_tmp")
                        nc.vector.tensor_scalar_mul(tmp[0:ow, :], dst, LRELU_CONV)
                        nc.vector.tensor_tensor(out=dst, in0=dst,
                                                in1=tmp[0:ow, :], op=ALU.max)
                        if li < 3:
                            nc.scalar.activation(out=nxa[j * P:j * P + ow, gsl],
                                                 in_=dst, func=AF.Copy, scale=2.0)

                def conv5(g):
                    """1024-ch conv + pooling for group g (after layer-3 epilogue)."""
                    gsl = slice(g * 512, (g + 1) * 512)
                    for j in range(8):
                        pc = ps_e.tile([P, 512], F32, tag="e_ps")
                        for ci, (lx, crow, fo) in enumerate(chains):
                            fsl = slice(fo + g * 512, fo + (g + 1) * 512)
                            nc.tensor.matmul(
                                out=pc, lhsT=w4c[ci][0:crow, j * P:(j + 1) * P],
                                rhs=xp[lx][0:crow, fsl],
                                start=(ci == 0), stop=(ci == len(chains) - 1))
                        # h pre-act; mean accumulators via two Act passes
                        hs = sbw.tile([P, 512], F32, tag="h_sb")
                        nc.scalar.activation(out=hs, in_=pc, func=AF.Identity,
                                             scale=s4[:, j:j + 1], bias=t4[:, j:j + 1],
                                             accum_out=mean_z[:, j, g:g + 1])
                        hr = sbw.tile([P, 512], F32, tag="lr_tmp")
                        nc.scalar.activation(out=hr, in_=hs, func=AF.Relu, scale=-1.0,
                                             accum_out=mean_r[:, j, g:g + 1])
                        # max-pool on pre-act h (lrelu applied to pooled value later)
                        pm = sbw.tile([P, 1], F32, tag="pmax")
                        nc.vector.tensor_reduce(out=pm, in_=hs, axis=AX.X, op=ALU.max)
                        if g == 0:
                            nc.vector.tensor_copy(out=p_cf[:, j:j + 1], in_=pm)
                        else:
                            nc.vector.tensor_tensor(out=p_cf[:, j:j + 1],
                                                    in0=p_cf[:, j:j + 1], in1=pm,
                                                    op=ALU.max)

                # --- per point-tile: S, top-k, idx wrap, gather, k-reduce
                # S of tile t+1 is emitted before tile t's top-k so the PE
                # computes it during the DVE scans (instead of idling behind
                # the selr matmuls that wait on the top-k).
                def emit_S(t):
                    s_sb = sbs.tile([P, N], F32, tag="s_sb", name="s_sb")
                    for q in range(4):
                        sl = slice(q * 512, (q + 1) * 512)
                        pq = ps_s.tile([P, 512], F32, tag="s_ps")
                        if not is3:
                            nc.tensor.matmul(out=pq,
                                             lhsT=xa[0:C + 1, t * P:(t + 1) * P],
                                             rhs=xb[0:C + 1, sl],
                                             start=True, stop=True)
                        else:
                            nc.tensor.matmul(out=pq,
                                             lhsT=xa[0:C, t * P:(t + 1) * P],
                                             rhs=xb[0:C, sl], start=True, stop=False)
                            nc.tensor.matmul(out=pq, lhsT=ones_row, rhs=nsq[:, sl],
                                             start=False, stop=True)
                        nc.scalar.activation(out=s_sb[:, sl], in_=pq, func=AF.Copy)
                    return s_sb

                pending = []
                epiq = []
                cur_m = [None]
                s_cur = emit_S(0)
                for t in range(NT):
                    if t % 4 == 0:
                        cur_m[0] = sbs.tile([P, 4 * O], F32, tag="m_grp", name="m_grp")
                    s_nxt = emit_S(t + 1) if t + 1 < NT else None
                    s_sb = s_cur
                    v24 = sbw.tile([P, 24], F32, tag="v24")
                    i24 = sbw.tile([P, 24], U32, tag="i24")
                    nc.vector.max(out=v24[:, 0:8], in_=s_sb)
                    nc.vector.max_index(out=i24[:, 0:8], in_max=v24[:, 0:8],
                                        in_values=s_sb)
                    nc.vector.match_replace(out=s_sb, in_to_replace=v24[:, 0:8],
                                            in_values=s_sb, imm_value=NEG)
                    nc.vector.max(out=v24[:, 8:16], in_=s_sb)
                    nc.vector.max_index(out=i24[:, 8:16], in_max=v24[:, 8:16],
                                        in_values=s_sb)
                    nc.vector.match_replace(out=s_sb, in_to_replace=v24[:, 8:16],
                                            in_values=s_sb, imm_value=NEG)
                    nc.vector.max(out=v24[:, 16:24], in_=s_sb)
                    nc.vector.max_index(out=i24[:, 16:24], in_max=v24[:, 16:24],
                                        in_values=s_sb)

                    idxf32 = sbw.tile([P, KNN], F32, tag="idxf32")
                    nc.vector.tensor_copy(out=idxf32, in_=i24[:, 0:KNN])
                    idxf = sbw.tile([P, KNN], F16, tag="idxf")
                    nc.vector.tensor_copy(out=idxf, in_=idxf32)
                    pw = ps_m.tile([P, 8 * KNN], F32, tag="m")
                    for g8 in range(8):
                        nc.tensor.matmul(
                            out=pw[:, :].rearrange("p (k g) -> p k g", g=8)[:, :, g8],
                            lhsT=selr[:, g8, :], rhs=idxf, start=True, stop=True,
                            skip_group_check=True)
                    w16 = sbw.tile([P, 8 * KNN], I16, tag="w16")
                    nc.vector.tensor_copy(out=w16, in_=pw)

                    # four quarter-gathers (5 neighbors each): each quarter's
                    # k-reduce overlaps the remaining quarters' gathers,
                    # shrinking the window where the DVE waits on the
                    # (Q7-bound) gather generation.
                    KH = KNN // 4
                    gts = []
                    for qg in range(4):
                        gtq = sbg.tile([P, KH, O], F32, tag=f"gather{qg}",
                                       name=f"gt{qg}")
                        nc.gpsimd.dma_gather(
                            out_ap=gtq[:, :, :], in_ap=u_dram[:, :],
                            idxs_ap=w16[:, qg * KH * 8:(qg + 1) * KH * 8],
                            num_idxs=P * KH, num_idxs_reg=P * KH,
                            elem_size=O, single_packet=False)
                        gts.append(gtq)
                    pending.append((t, gts, cur_m[0]))

                    def flush_one():
                        tk, gtsk, mgk = pending.pop(0)
                        msl = mgk[:, (tk % 4) * O:(tk % 4 + 1) * O]
                        nc.vector.tensor_reduce(
                            out=msl,
                            in_=gtsk[0][:, :, :].rearrange("p k o -> p o k"),
                            axis=AX.X, op=ALU.max)
                        for qg in range(1, 4):
                            mtmp = sbw.tile([P, O], F32, tag="mtmp")
                            nc.vector.tensor_reduce(
                                out=mtmp,
                                in_=gtsk[qg][:, :, :].rearrange("p k o -> p o k"),
                                axis=AX.X, op=ALU.max)
                            nc.vector.tensor_tensor(out=msl, in0=msl, in1=mtmp,
                                                    op=ALU.max)
                        if tk % 4 == 3:
                            epiq.append((tk // 4, mgk))

                    def drain_epis(now):
                        while epiq and (now or epiq[0][0] * 4 + 6 <= t):
                            g, mgk = epiq.pop(0)
                            epilogue(g, mgk)
                            if li == 3:
                                conv5(g)

                    # lag-1 software pipeline: k-reduce of tile t-1 issues after
                    # tile t's top-k, so the DVE never stalls on the gather.
                    # Epilogues are emitted 2+ tiles later still, so their PE ops
                    # never block upcoming S matmuls on not-yet-passed DVE points.
                    if len(pending) > 1:
                        flush_one()
                    drain_epis(False)
                    if t == NT - 1:
                        while pending:
                            flush_one()
                        drain_epis(True)
                    s_cur = s_nxt
                if li < 3:
                    # ones row of next xA (if next layer has aug row)
                    if CONV[li + 1][1] < P:
                        nc.sync.dma_start(
                            out=nxa[CONV[li + 1][1]:CONV[li + 1][1] + 1, :],
                            in_=t_w["onesN"][:, :])
                if dbg:
                    nc.sync.dma_start(out=t_out[f"dbg_x{li}"][:, :],
                                      in_=xp[li][0:min(O, P), :])
            if li < 3:
                xa = nxa
                xb = xp[li]
                if CONV[li + 1][1] < P:
                    nsq = xp[li][CONV[li + 1][1]:CONV[li + 1][1] + 1, :]
                else:
                    nsq = sbx.tile([1, N], F32, tag="nsq3", name="nsq3")

        # =================== finish pooling ===================
        # mean = (sum_z - 0.8 * sum_relu(-z)) / N ; p_cf[:, 8+j]
        with tc.tile_pool(name="sb_pool", bufs=1) as sbp:
            mz = sbp.tile([P, 8], F32)
            mr = sbp.tile([P, 8], F32)
            nc.vector.tensor_reduce(out=mz, in_=mean_z[:, :, :], axis=AX.X, op=ALU.add)
            nc.vector.tensor_reduce(out=mr, in_=mean_r[:, :, :], axis=AX.X, op=ALU.add)
            # sum lrelu(z) = sum z + (1 - alpha) * sum relu(-z)
            nc.vector.tensor_scalar_mul(mr, mr, 1.0 - LRELU_CONV)
            nc.vector.tensor_add(out=p_cf[:, 8:16], in0=mz, in1=mr)
            nc.vector.tensor_scalar_mul(p_cf[:, 8:16], p_cf[:, 8:16], 1.0 / N)
            # lrelu on max-pooled columns (monotone: lrelu(max) = max(lrelu))
            t8 = sbp.tile([P, 8], F32)
            nc.vector.tensor_scalar_mul(t8, p_cf[:, 0:8], LRELU_CONV)
            nc.vector.tensor_tensor(out=p_cf[:, 0:8], in0=p_cf[:, 0:8], in1=t8,
                                    op=ALU.max)
            if dbg:
                nc.sync.dma_start(out=t_out["dbg_p"][:, :], in_=p_cf[:, :])

        mstack.close()
        # =================== MLP head (broadcast + DVE dot-products) ==========
        with ExitStack() as hctx:
            sb = hctx.enter_context(tc.tile_pool(name="sb_head", bufs=1))
            sbwh = hctx.enter_context(tc.tile_pool(name="sbw_head", bufs=2))

            def lin(name, src_col, incols, w_dram, out_dim, alpha):
                in_dim = P * incols
                och = (out_dim + P - 1) // P
                orows = min(P, out_dim)
                bcast = sb.tile([P, in_dim], F32, tag=f"{name}_bc")
                for j in range(incols):
                    pT = ps_m.tile([1, P], F32, tag="m")
                    nc.tensor.transpose(out=pT, in_=src_col[:, j:j + 1],
                                        identity=ident)
                    rowj = sbwh.tile([1, P], F32, tag="hd_row")
                    nc.scalar.activation(out=rowj, in_=pT, func=AF.Copy)
                    pb = ps_m.tile([P, P], F32, tag="m")
                    nc.tensor.matmul(out=pb, lhsT=ones_row, rhs=rowj,
                                     start=True, stop=True)
                    nc.scalar.activation(out=bcast[:, j * P:(j + 1) * P], in_=pb,
                                         func=AF.Copy)
                dst = sb.tile([P, och], F32, tag=f"{name}_out")
                for ot in range(och):
                    orw = min(P, out_dim - ot * P)
                    wsb = sbwh.tile([P, in_dim], F32, tag=f"{name}_w")
                    nc.sync.dma_start(out=wsb[0:orw, :],
                                      in_=w_dram[ot * P:ot * P + orw, :])
                    prod = sbwh.tile([P, in_dim], F32, tag=f"{name}_prod")
                    nc.vector.tensor_mul(prod[0:orw, :], wsb[0:orw, :], bcast[0:orw, :])
                    nc.vector.tensor_reduce(out=dst[0:orw, ot:ot + 1],
                                            in_=prod[0:orw, :], axis=AX.X, op=ALU.add)
                if alpha is not None:
                    tmp = sbwh.tile([P, och], F32, tag=f"{name}_tmp")
                    nc.vector.tensor_scalar_mul(tmp[0:orows, :], dst[0:orows, :], alpha)
                    nc.vector.tensor_tensor(out=dst[0:orows, :], in0=dst[0:orows, :],
                                            in1=tmp[0:orows, :], op=ALU.max)
                return dst

            y1 = lin("y1", p_cf, 16, t_w["L1"], 512, LRELU_HEAD)
            y2 = lin("y2", y1, 4, t_w["L2"], 256, LRELU_HEAD)
            y3 = lin("y3", y2, 2, t_w["L3"], 128, LRELU_HEAD)
            y4 = lin("y4", y3, 1, t_w["L4"], 2, None)
            osb = sb.tile([2, 1], F32, tag="out_sb")
            nc.vector.tensor_copy(out=osb, in_=y4[0:2, 0:1])
            nc.sync.dma_start(out=t_out["out"][:, :], in_=osb)


_PROG_CACHE = {}


def _build(dbg=False):
    key = ("v2", dbg)
    if key in _PROG_CACHE:
        return _PROG_CACHE[key]
    nc = bacc.Bacc("TRN2", target_bir_lowering=False, debug=False, num_devices=B)
    t_in = {"feat_xyz": nc.declare_dram_parameter("feat_xyz", [N, 3], F32,
                                                  isOutput=False)}
    t_w = {}
    for li, (O, C) in enumerate(CONV):
        t_w[f"wlT{li}"] = nc.declare_dram_parameter(f"wlT{li}", [C, O], F32,
                                                    isOutput=False)
        t_w[f"wvT{li}"] = nc.declare_dram_parameter(f"wvT{li}", [C, O], F32,
                                                    isOutput=False)
        t_w[f"bns{li}"] = nc.declare_dram_parameter(f"bns{li}", [O, 1], F32,
                                                    isOutput=False)
        t_w[f"bnt{li}"] = nc.declare_dram_parameter(f"bnt{li}", [O, 1], F32,
                                                    isOutput=False)
        t_w[f"Utab{li}"] = nc.dram_tensor(f"Utab{li}", [N, O], F32)
    t_w["w4t"] = nc.declare_dram_parameter("w4t", [512, 1024], F32,
                                           isOutput=False)
    t_w["onesN"] = nc.declare_dram_parameter("onesN", [1, N], F32, isOutput=False)
    t_w["bns4"] = nc.declare_dram_parameter("bns4", [1024, 1], F32, isOutput=False)
    t_w["bnt4"] = nc.declare_dram_parameter("bnt4", [1024, 1], F32, isOutput=False)
    for j, (o, c) in enumerate(LIN):
        t_w[f"L{j+1}"] = nc.declare_dram_parameter(f"L{j+1}", [o, c], F32,
                                                   isOutput=False)
    t_out = {"out": nc.declare_dram_parameter("out", [2, 1], F32, isOutput=True)}
    if dbg:
        for li in range(4):
            O = CONV[li][0]
            sh = [P, 2 * N] if O == 256 else [O, N]
            t_out[f"dbg_x{li}"] = nc.declare_dram_parameter(f"dbg_x{li}", sh,
                                                            F32, isOutput=True)
        t_out["dbg_p"] = nc.declare_dram_parameter("dbg_p", [P, 16], F32,
                                                   isOutput=True)

    with tile.TileContext(nc) as tc:
        _emit(nc, tc, t_in, t_w, t_out, dbg)
    nc.compile()
    _PROG_CACHE[key] = nc
    return nc


def _make_in_maps(inputs):
    f32 = lambda a: np.ascontiguousarray(np.asarray(a, np.float32))
    feat = f32(inputs["feat_xyz"])
    common = {}
    for li, (O, C) in enumerate(CONV):
        W = f32(inputs[f"W{li}"])
        wl, wr = W[:, :C], W[:, C:]
        common[f"wlT{li}"] = f32(wl.T)
        common[f"wvT{li}"] = f32((wr - wl).T)
        g, b, m, v = (f32(inputs[f"{n}{li}"]) for n in "gbmv")
        s = g / np.sqrt(v + EPS)
        common[f"bns{li}"] = f32(s.reshape(-1, 1))
        common[f"bnt{li}"] = f32((b - m * s).reshape(-1, 1))
    common["w4t"] = np.ascontiguousarray(f32(inputs["W4"]).T)   # [512, 1024]
    g, b, m, v = (f32(inputs[f"{n}4"]) for n in "gbmv")
    s = g / np.sqrt(v + EPS)
    common["bns4"] = f32(s.reshape(-1, 1))
    common["bnt4"] = f32((b - m * s).reshape(-1, 1))
    common["onesN"] = np.ones((1, N), np.float32)
    for j in range(1, 5):
        common[f"L{j}"] = f32(inputs[f"L{j}"])
    return [dict(common, feat_xyz=np.ascontiguousarray(feat[b])) for b in range(B)]


def run(inputs, dbg=False, trace=False, **kw):
    nc = _build(dbg)
    in_maps = _make_in_maps(inputs)
    return run_bass_kernel_spmd(nc, in_maps, list(range(B)), trace=trace, **kw)


def kernel(**inputs):
    res = run(inputs).results
    out = np.stack([res[b]["out"][:, 0] for b in range(B)], axis=0)
    return out.astype(np.float32)
